# revision 32
# baseline (speedup 1.0000x reference)
"""Causal single-head attention (B=4, S=2048, E=1024, D=128) on 8 trn2 cores.

Sharding: 2 cores per batch, role-balanced causal split: each core computes
attention for 1024 query rows of its batch; the host permutes 512-row blocks
per core role so both roles run one uniform SPMD program:

  role 0: perm = [0:512 | 512:1024 | 1536:2048 | 1024:1536]
  role 1: perm = [512:1024 | 0:512 | 1024:1536 | 1536:2048]

Queries: permuted positions [0,512) (slot 0, key extent 8 tiles) and
[1024,1536) (slot 1, extent 16 tiles).  Masking is free/cheap:
  - all-or-nothing units: role-baked bias on the exp (exp(s*scale-30000)=0)
  - true-diagonal units (slot0 j0-3, slot1 j8-11, same for both roles):
    gpsimd affine_select (keep where s - p - 128*(j%4) >= 0).

Device program (fp16 operands, f32 PSUM):
  xT arrives HOST-TRANSPOSED as [128p, 8ch, 2048s] fp16 (no PE transposes)
  K^T[tb] = sum_ch wk[ch].T @ xT[ch, tb]  (+bk on DVE)  -> fp16
  V[t,d]  = per key-tile sum_ch xT[ch, t128].T @ wv[ch] -> fp16
  Q^T[slot] = sum_ch wq[ch].T @ xT[ch, qcols] (+bq on DVE) -> fp16
  per slot, unit pair: st pair -> one exp over [128,1024] (bias gb) fp16
  diag units: affine_select ; racc (+)= pt halves  [DVE fp16]
  ot[d,q] += v_j.T @ pt  [PE] ; rs = ones.T@racc + ones.T@pt(last pair)
  host: out = (ot/rs).T + bv

Emission is software-pipelined: attention PV lags one unit behind st/exp,
and projection work for later tb blocks is interleaved between attention
instructions so the in-order PE never stalls on the exp round-trip.
"""

import math

import numpy as np

B, S, E, D = 4, 2048, 1024, 128
P = 128
EC = E // P          # 8 E-chunks
NT = S // P          # 16 key tiles
QB_NT = (8, 16)      # key-tile extent per slot
SCALE = 1.0 / math.sqrt(D)
NEG = -30000.0

# true-diagonal units (same relative triangle for both roles)
DIAG = {(0, j) for j in range(4)} | {(1, j) for j in range(8, 12)}


def _role_perm(role):
    a = np.arange
    if role == 0:
        blocks = [a(0, 512), a(512, 1024), a(1536, 2048), a(1024, 1536)]
    else:
        blocks = [a(512, 1024), a(0, 512), a(1024, 1536), a(1536, 2048)]
    return np.concatenate(blocks)


def _build_nc():
    from contextlib import ExitStack

    import concourse.bass as bass
    import concourse.tile as tile
    from concourse import bacc, mybir

    f16 = mybir.dt.float16
    f32 = mybir.dt.float32
    f8 = mybir.dt.float8e4
    DR = mybir.MatmulPerfMode.DoubleRow
    AF = mybir.ActivationFunctionType

    nc = bacc.Bacc("TRN2", target_bir_lowering=False, debug=False)

    # host-transposed x: [p, ch, s] fp16
    xt_in = nc.dram_tensor("xt", [P, EC, S], f16, kind="ExternalInput")
    w_in = {
        n: nc.dram_tensor(n, [P, EC, D], f16, kind="ExternalInput")
        for n in ("wq", "wk", "wv")
    }
    # cst32: col 0 = bq, col 1 = bk, cols 2..33 = gb (exp bias per slot*16+j)
    cst32_in = nc.dram_tensor("cst32", [P, 34], f32, kind="ExternalInput")
    # ones column (fp16) for the rowsum matmuls
    cst8_in = nc.dram_tensor("cst8", [P, 1], f16, kind="ExternalInput")
    ot_out = nc.dram_tensor("ot", [P, 1024], f32, kind="ExternalOutput")
    rs_out = nc.dram_tensor("rs", [1, 1024], f32, kind="ExternalOutput")

    with tile.TileContext(nc) as tc, ExitStack() as ctx:
        consts = ctx.enter_context(tc.tile_pool(name="consts", bufs=1))
        xt_pool = ctx.enter_context(tc.tile_pool(name="xt", bufs=4))
        pt_pool = ctx.enter_context(tc.tile_pool(name="pt", bufs=8))
        out_pool = ctx.enter_context(tc.tile_pool(name="outp", bufs=1))
        pj_psum = ctx.enter_context(tc.tile_pool(name="pjp", bufs=1, space="PSUM"))
        vv_psum = ctx.enter_context(tc.tile_pool(name="vvp", bufs=1, space="PSUM"))
        st_psum = ctx.enter_context(tc.tile_pool(name="stp", bufs=2, space="PSUM"))
        ot_psum = ctx.enter_context(tc.tile_pool(name="otp", bufs=1, space="PSUM"))

        # ---- DMA plan: one HWDGE queue (sync) carries everything in
        # priority order; the first K matmul only needs wk chunk 0 (on the
        # scalar queue, racing in parallel) + xt0 chunk 0.
        w_sb = {}
        for n in ("wk", "wv", "wq"):
            w_sb[n] = consts.tile([P, EC, D], f16, name=f"w_{n}")
        xt_tiles = {}
        for tb in range(4):
            xt_tiles[tb] = xt_pool.tile([P, EC, 512], f16, tag="xt", name=f"xt_{tb}")
        cst32 = consts.tile([P, 34], f32)
        cst8 = consts.tile([P, 1], f16)

        def ld(q, sb, dram, ch0, ch1, col0=None, col1=None):
            if col0 is None:
                q.dma_start(out=sb[:, ch0:ch1], in_=dram[:, ch0:ch1])
            else:
                q.dma_start(
                    out=sb[:, ch0:ch1, :], in_=dram[:, ch0:ch1, col0:col1]
                )

        ld(nc.scalar, w_sb["wk"], w_in["wk"], 0, 2)            # wk c0-1
        ld(nc.sync, xt_tiles[0], xt_in, 0, 2, 0, 512)          # xt0 c0-1
        ld(nc.sync, w_sb["wk"], w_in["wk"], 2, EC)             # wk rest
        ld(nc.sync, xt_tiles[0], xt_in, 2, 4, 0, 512)          # xt0 c2-3
        ld(nc.sync, xt_tiles[0], xt_in, 4, 6, 0, 512)          # xt0 c4-5
        ld(nc.sync, xt_tiles[0], xt_in, 6, EC, 0, 512)         # xt0 c6-7
        nc.sync.dma_start(out=cst32[:], in_=cst32_in[:, :])    # biases + gb
        ld(nc.sync, w_sb["wq"], w_in["wq"], 0, EC)
        for h in range(2):                                     # xt1
            ld(nc.sync, xt_tiles[1], xt_in, h * 4, (h + 1) * 4, 512, 1024)
        ld(nc.sync, w_sb["wv"], w_in["wv"], 0, EC)
        nc.sync.dma_start(out=cst8[:], in_=cst8_in[:, :])
        for tb in (2, 3):
            for h in range(2):
                ld(nc.sync, xt_tiles[tb], xt_in, h * 4, (h + 1) * 4,
                   tb * 512, (tb + 1) * 512)

        bq, bk = cst32[:, 0:1], cst32[:, 1:2]
        ones = cst8[:, :]

        kt_tiles = {}
        qt_tiles = {}
        v_big = consts.tile([P, NT, D], f16, name="v_big")

        def proj_k_gen(tb):
            xt = xt_tiles[tb]
            pp = pj_psum.tile([P, 512], f32, tag="pj")
            for c in range(EC):
                nc.tensor.matmul(
                    pp[:], w_sb["wk"][:, c, :], xt[:, c, :],
                    start=(c == 0), stop=(c == EC - 1),
                )
                if c % 2 == 1:
                    yield
            kt = consts.tile([P, 512], f16, name=f"kt_{tb}")
            nc.vector.tensor_scalar_add(kt[:], pp[:], bk)
            kt_tiles[tb] = kt
            yield

        def proj_v_gen(tb):
            xt = xt_tiles[tb]
            for jp in range(2):
                vp = vv_psum.tile([P, 2, D], f32, tag="vv")
                for h in range(2):
                    jl = jp * 2 + h
                    for c in range(EC):
                        nc.tensor.matmul(
                            vp[:, h, :],
                            xt[:, c, jl * P : (jl + 1) * P],
                            w_sb["wv"][:, c, :],
                            start=(c == 0), stop=(c == EC - 1),
                        )
                    yield
                nc.vector.tensor_copy(
                    v_big[:, tb * 4 + jp * 2 : tb * 4 + jp * 2 + 2, :], vp[:]
                )

        def proj_q_gen(slot, tb):
            # slot0 queries = permuted cols 0..511 (= tb0); slot1 = cols
            # 1024..1535 (= tb2) — the full tb tile is exactly the slot.
            xt = xt_tiles[tb]
            pp = pj_psum.tile([P, 512], f32, tag="pj")
            for c in range(EC):
                nc.tensor.matmul(
                    pp[:], w_sb["wq"][:, c, :], xt[:, c, :],
                    start=(c == 0), stop=(c == EC - 1),
                )
                if c % 2 == 1:
                    yield
            qt = consts.tile([P, 512], f16, name=f"qt_{slot}")
            nc.vector.tensor_scalar_add(qt[:], pp[:], bq)
            qt_tiles[slot] = qt
            yield

        ot_sb = out_pool.tile([P, 1024], f32)
        rs_sb = out_pool.tile([1, 1024], f32)

        # Pair-level attention: each pair (2k, 2k+1) shares one [128,1024]
        # PSUM score tile, one paired exp, and fp8 DoubleRow PV/rowsum
        # matmuls. Diagonal (Pool-masked) pairs are interleaved between
        # plain pairs so the Pool select never gates two pairs in a row;
        # kt3-dependent pairs (slot1 units 12..15) stay last.
        PAIR_ORDER = {
            0: [0, 2, 1, 3],
            1: [0, 4, 1, 5, 2, 3, 6, 7],
        }

        def attn_slot_gen(slot):
            n_pr = QB_NT[slot] // 2
            qt = qt_tiles[slot]
            ot = ot_psum.tile([P, 512], f32, tag="ot")
            rp = ot_psum.tile([1, 512], f32, tag="rs")
            racc = out_pool.tile([P, 512], f16, name=f"racc_{slot}")

            def emit_pv(pos, pr, ptp):
                for h in (0, 1):
                    nc.tensor.matmul(
                        ot[:], v_big[:, 2 * pr + h, :], ptp[:, h, :],
                        start=(pos == 0 and h == 0),
                        stop=(pos == n_pr - 1 and h == 1),
                    )
                if pos == n_pr - 2:
                    # racc finalized; its rowsum overlaps the last pair's exp
                    nc.tensor.matmul(rp[:], ones, racc[:], start=True, stop=False)

            pend = []
            for pos, pr in enumerate(PAIR_ORDER[slot]):
                j0 = 2 * pr
                last = pos == n_pr - 1
                stp = st_psum.tile([P, 2, 512], f32, tag="st")
                for h in (0, 1):
                    j = j0 + h
                    nc.tensor.matmul(
                        stp[:, h, :],
                        kt_tiles[j // 4][:, (j % 4) * P : (j % 4 + 1) * P],
                        qt[:], start=True, stop=True,
                    )
                ptp = pt_pool.tile([P, 2, 512], f16, tag="pt")
                gbias = cst32[:, 2 + slot * 16 + j0 : 3 + slot * 16 + j0]
                if not last:
                    nc.scalar.activation(
                        out=ptp[:], in_=stp[:], func=AF.Exp, scale=SCALE, bias=gbias,
                    )
                else:
                    # split the final exp so its PV can start after the first
                    # half, shortening the slot's drain tail
                    for h in (0, 1):
                        nc.scalar.activation(
                            out=ptp[:, h, :], in_=stp[:, h, :], func=AF.Exp,
                            scale=SCALE, bias=gbias,
                        )
                for h in (0, 1):
                    j = j0 + h
                    if (slot, j) in DIAG:
                        # keep pt[p, s] where s - p - (j%4)*128 >= 0 else 0
                        nc.gpsimd.affine_select(
                            out=ptp[:, h, :],
                            in_=ptp[:, h, :],
                            pattern=[[1, 512]],
                            compare_op=mybir.AluOpType.is_ge,
                            fill=0.0,
                            base=-(j % 4) * P,
                            channel_multiplier=-1,
                        )
                if pos < n_pr - 1:
                    if pos == 0:
                        nc.vector.tensor_copy(racc[:], ptp[:, 0, :])
                        nc.vector.tensor_add(racc[:], racc[:], ptp[:, 1, :])
                    else:
                        nc.vector.tensor_add(racc[:], racc[:], ptp[:, 0, :])
                        nc.vector.tensor_add(racc[:], racc[:], ptp[:, 1, :])
                pend.append((pos, pr, ptp))
                yield
                if len(pend) > 2:
                    emit_pv(*pend.pop(0))
                yield
            while pend:
                pos, pr, ptp = pend.pop(0)
                if pos < n_pr - 1:
                    emit_pv(pos, pr, ptp)
                else:
                    for h in (0, 1):
                        nc.tensor.matmul(
                            ot[:], v_big[:, 2 * pr + h, :], ptp[:, h, :],
                            start=(pos == 0 and h == 0), stop=(h == 1),
                        )
                        nc.tensor.matmul(
                            rp[:], ones[:, 0:1], ptp[:, h, :],
                            start=False, stop=(h == 1),
                        )
            # split output copies across DVE/Act, DMA each piece when ready
            nc.vector.tensor_copy(ot_sb[:, slot * 512 : slot * 512 + 256], ot[:, 0:256])
            nc.scalar.copy(ot_sb[:, slot * 512 + 256 : (slot + 1) * 512], ot[:, 256:512])
            nc.vector.tensor_copy(rs_sb[0:1, slot * 512 : (slot + 1) * 512], rp[:])
            nc.sync.dma_start(
                out=ot_out[:, slot * 512 : slot * 512 + 256],
                in_=ot_sb[:, slot * 512 : slot * 512 + 256],
            )
            nc.scalar.dma_start(
                out=ot_out[:, slot * 512 + 256 : (slot + 1) * 512],
                in_=ot_sb[:, slot * 512 + 256 : (slot + 1) * 512],
            )
            nc.sync.dma_start(
                out=rs_out[:, slot * 512 : (slot + 1) * 512],
                in_=rs_sb[0:1, slot * 512 : (slot + 1) * 512],
            )

        # ---- schedule: interleave attention with later projections ----
        def drain(gen):
            for _ in gen:
                pass

        def interleave(main, filler, ratio=1):
            for _ in main:
                for _ in range(ratio):
                    next(filler, None)

        def chain(*gens):
            for g in gens:
                yield from g

        drain(proj_k_gen(0))
        drain(proj_q_gen(0, 0))
        f1 = chain(
            proj_k_gen(1), proj_v_gen(0), proj_v_gen(1),
            proj_k_gen(2), proj_q_gen(1, 2),
        )
        interleave(attn_slot_gen(0), f1, ratio=3)
        drain(f1)
        f2 = chain(proj_v_gen(2), proj_k_gen(3), proj_v_gen(3))
        interleave(attn_slot_gen(1), f2)
        drain(f2)

    nc.compile()
    return nc


_NC_CACHE = {}


def _get_nc():
    if "nc" not in _NC_CACHE:
        _NC_CACHE["nc"] = _build_nc()
    return _NC_CACHE["nc"]


def _get_runner():
    """Cached PJRT executable (same lowering as bass2jax.run_bass_via_pjrt,
    but the jitted function is built once and reused across calls)."""
    if "runner" in _NC_CACHE:
        return _NC_CACHE["runner"]

    import jax
    from jax.sharding import Mesh, PartitionSpec
    from jax.experimental.shard_map import shard_map
    from concourse import bass2jax, mybir

    nc = _get_nc()
    bass2jax.install_neuronx_cc_hook()

    partition_name = nc.partition_id_tensor.name if nc.partition_id_tensor else None
    in_names, out_names, out_avals = [], [], []
    for alloc in nc.m.functions[0].allocations:
        if not isinstance(alloc, mybir.MemoryLocationSet):
            continue
        name = alloc.memorylocations[0].name
        if alloc.kind == "ExternalInput":
            if name != partition_name:
                in_names.append(name)
        elif alloc.kind == "ExternalOutput":
            out_names.append(name)
            out_avals.append(
                jax.core.ShapedArray(tuple(alloc.tensor_shape), mybir.dt.np(alloc.dtype))
            )
    n_params = len(in_names)
    all_names = in_names + out_names
    if partition_name is not None:
        all_names = all_names + [partition_name]

    def _body(*args):
        operands = list(args)
        if partition_name is not None:
            operands.append(bass2jax.partition_id_tensor())
        outs = bass2jax._bass_exec_p.bind(
            *operands,
            out_avals=tuple(out_avals),
            in_names=tuple(all_names),
            out_names=tuple(out_names),
            lowering_input_output_aliases=(),
            sim_require_finite=True,
            sim_require_nnan=True,
            nc=nc,
        )
        return tuple(outs)

    devices = jax.devices()[:8]
    mesh = Mesh(np.asarray(devices), ("core",))
    sharded = jax.jit(
        shard_map(
            _body,
            mesh=mesh,
            in_specs=(PartitionSpec("core"),) * (n_params + len(out_names)),
            out_specs=(PartitionSpec("core"),) * len(out_names),
            check_rep=False,
        ),
        donate_argnums=tuple(range(n_params, n_params + len(out_names))),
        keep_unused=True,
    )
    runner = {
        "sharded": sharded,
        "in_names": in_names,
        "out_names": out_names,
        "out_avals": out_avals,
    }
    _NC_CACHE["runner"] = runner
    return runner


def _prep_in_concat(x, wq, bq, wk, bk, wv, bv):
    """Per-core in_maps, concatenated along axis 0 for shard_map."""
    x = np.asarray(x, dtype=np.float32)

    if "perm" not in _NC_CACHE:
        _NC_CACHE["perm"] = [_role_perm(0), _role_perm(1)]
    perms = _NC_CACHE["perm"]

    import ml_dtypes

    f8 = ml_dtypes.float8_e4m3fn

    def pack_w(w):
        # [E, D] -> [p, ch, d] fp16
        return np.ascontiguousarray(
            np.asarray(w, np.float32).reshape(EC, P, D).transpose(1, 0, 2)
        ).astype(np.float16)

    w16 = {"wq": pack_w(wq), "wk": pack_w(wk), "wv": pack_w(wv)}
    _NC_CACHE["bv"] = np.asarray(bv, np.float32)

    # cst32: bq, bk, gb (exp bias: -30000 on role's all-invalid units)
    cst32 = []
    for role in (0, 1):
        c = np.zeros((P, 34), np.float32)
        c[:, 0] = np.asarray(bq, np.float32)
        c[:, 1] = np.asarray(bk, np.float32)
        if role == 0:
            c[:, 2 + 4 : 2 + 8] = NEG            # slot0 j4..7
        else:
            c[:, 2 + 16 + 12 : 2 + 16 + 16] = NEG  # slot1 j12..15
        cst32.append(c)

    c8 = np.ones((P, 1), np.float16)

    # per-batch transposed x, then per-core column gather + fp16 + chunk layout
    xt_cores = []
    for b in range(B):
        xbT = np.ascontiguousarray(x[b].T)  # [E, S]
        for role in (0, 1):
            xg = xbT[:, perms[role]].astype(np.float16)      # [E, S]
            xt_cores.append(
                np.ascontiguousarray(xg.reshape(EC, P, S).transpose(1, 0, 2))
            )

    runner = _get_runner()
    concat = {
        "xt": np.concatenate(xt_cores, axis=0),
        "cst32": np.concatenate([cst32[c % 2] for c in range(8)], axis=0),
        "cst8": np.concatenate([c8] * 8, axis=0),
    }
    for n, v in w16.items():
        concat[n] = np.concatenate([v] * 8, axis=0)
    return [concat[n] for n in runner["in_names"]]


def _run_concat(concat_in):
    runner = _get_runner()
    zeros = [
        np.zeros((8 * a.shape[0], *a.shape[1:]), a.dtype) for a in runner["out_avals"]
    ]
    out_arrs = runner["sharded"](*concat_in, *zeros)
    ot = np.asarray(out_arrs[runner["out_names"].index("ot")]).reshape(8, P, 1024)
    rs = np.asarray(out_arrs[runner["out_names"].index("rs")]).reshape(8, 1024)
    return ot, rs


def _assemble(ot, rs):
    perms = _NC_CACHE["perm"]
    bv = _NC_CACHE["bv"]
    out = np.empty((B, S, D), dtype=np.float32)
    for c in range(8):
        b, role = divmod(c, 2)
        perm = perms[role]
        for slot, qpos0 in ((0, 0), (1, 1024)):
            otT = ot[c][:, slot * 512 : (slot + 1) * 512]          # [D, 512]
            rsq = rs[c][slot * 512 : (slot + 1) * 512]             # [512]
            out[b, perm[qpos0 : qpos0 + 512]] = (otT / rsq[None, :]).T + bv[None, :]
    return out


def kernel(x, wq, bq, wk, bk, wv, bv):
    concat_in = _prep_in_concat(x, wq, bq, wk, bk, wv, bv)
    ot, rs = _run_concat(concat_in)
    return _assemble(ot, rs)


def bench(x, wq, bq, wk, bk, wv, bv, iters=20):
    """Per-launch wall time with device-resident inputs (upper bound on HW exec)."""
    import time

    import jax

    runner = _get_runner()
    concat_in = _prep_in_concat(x, wq, bq, wk, bk, wv, bv)
    dev_in = [jax.device_put(a) for a in concat_in]
    for a in dev_in:
        a.block_until_ready()
    times = []
    for _ in range(iters):
        zeros = [
            np.zeros((8 * a.shape[0], *a.shape[1:]), a.dtype)
            for a in runner["out_avals"]
        ]
        t0 = time.perf_counter()
        out = runner["sharded"](*dev_in, *zeros)
        for a in out:
            a.block_until_ready()
        times.append(time.perf_counter() - t0)
    return times


# revision 33
# speedup vs baseline: 1.0551x; 1.0551x over previous
"""Causal single-head attention (B=4, S=2048, E=1024, D=128) on 8 trn2 cores.

KEY-SPLIT sharding: 2 cores per batch. Core role r owns the key tiles with
nat index == r (mod 2) (8 of 16 128-row tiles) and computes K/V only for
those; BOTH cores process ALL 2048 queries against their own keys
(flash-style partials), and the host combines:
    out = (otA + otB) / (rsA + rsB)  (+bv, transpose)

xt column layout per core (pos space): cols 0..1023 = own key tiles
ascending, cols 1024..2047 = the other role's tiles ascending. Query
slot s = pos-tiles {own 2s, own 2s+1, other 2s, other 2s+1} (512 queries),
so slot s needs exactly the local key tiles 0..2s+1 for BOTH roles:
20 score units per core, no padding waste. Only the last key-tile pair of
each slot can straddle the diagonal; it is masked with a position-compare
STT (qpos >= tpos, host tables, role-baked).

Device program (fp16 operands, f32 PSUM, exp pairs over [128,1024]):
  K^T, V from own xt cols; Q^T per slot from (own pair | other pair) cols
  slot s, key pair u: st[t,q] pair -> exp pair -> (last pair: STT mask)
  racc (+)= pt halves [DVE];  ot[d,q] += v.T @ pt  [PE]
  rs = ones.T@racc + direct ones.T@pt for the last pair
"""

import math

import numpy as np

B, S, E, D = 4, 2048, 1024, 128
P = 128
EC = E // P          # 8 E-chunks
NK = 8               # own key tiles per core
SCALE = 1.0 / math.sqrt(D)


def _pos_to_nat(role):
    """pos-tile -> nat-tile mapping per role."""
    own = [2 * t + role for t in range(NK)]
    other = [2 * t + 1 - role for t in range(NK)]
    return np.array(own + other)


def _build_nc():
    from contextlib import ExitStack

    import concourse.bass as bass
    import concourse.tile as tile
    from concourse import bacc, mybir

    f16 = mybir.dt.float16
    f32 = mybir.dt.float32
    AF = mybir.ActivationFunctionType

    nc = bacc.Bacc("TRN2", target_bir_lowering=False, debug=False)

    xt_in = nc.dram_tensor("xt", [P, EC, S], f16, kind="ExternalInput")
    w_in = {
        n: nc.dram_tensor(n, [P, EC, D], f16, kind="ExternalInput")
        for n in ("wq", "wk", "wv")
    }
    # cst32: col0 = bq, col1 = bk
    cst32_in = nc.dram_tensor("cst32", [P, 2], f32, kind="ExternalInput")
    # ones column + tpos (own key nat positions per local tile)
    cst16_in = nc.dram_tensor("cst16", [P, 1 + NK], f16, kind="ExternalInput")
    # qpos: nat position of the query in each SLOT column (replicated rows)
    qpos_in = nc.dram_tensor("qpos", [P, S], f16, kind="ExternalInput")
    ot_out = nc.dram_tensor("ot", [P, S], f32, kind="ExternalOutput")
    rs_out = nc.dram_tensor("rs", [1, S], f32, kind="ExternalOutput")

    with tile.TileContext(nc) as tc, ExitStack() as ctx:
        consts = ctx.enter_context(tc.tile_pool(name="consts", bufs=1))
        xt_pool = ctx.enter_context(tc.tile_pool(name="xt", bufs=4))
        pt_pool = ctx.enter_context(tc.tile_pool(name="pt", bufs=6))
        out_pool = ctx.enter_context(tc.tile_pool(name="outp", bufs=1))
        pj_psum = ctx.enter_context(tc.tile_pool(name="pjp", bufs=1, space="PSUM"))
        vv_psum = ctx.enter_context(tc.tile_pool(name="vvp", bufs=1, space="PSUM"))
        st_psum = ctx.enter_context(tc.tile_pool(name="stp", bufs=2, space="PSUM"))
        ot_psum = ctx.enter_context(tc.tile_pool(name="otp", bufs=1, space="PSUM"))

        w_sb = {}
        for n in ("wk", "wv", "wq"):
            w_sb[n] = consts.tile([P, EC, D], f16, name=f"w_{n}")
        xt_tiles = {}   # 4 column-quarters of 512
        for qb in range(4):
            xt_tiles[qb] = xt_pool.tile([P, EC, 512], f16, tag="xt", name=f"xt_{qb}")
        cst32 = consts.tile([P, 2], f32)
        cst16 = consts.tile([P, 1 + NK], f16)
        qpos_sb = consts.tile([P, S], f16)

        def ld(q, sb, dram, ch0, ch1, col0=None, col1=None):
            if col0 is None:
                q.dma_start(out=sb[:, ch0:ch1], in_=dram[:, ch0:ch1])
            else:
                q.dma_start(out=sb[:, ch0:ch1, :], in_=dram[:, ch0:ch1, col0:col1])

        # priority order: wk + own K cols, then Q weights + other-block
        # cols + slot-0 tables (the attention cascade starts at slot 0)
        ld(nc.scalar, w_sb["wk"], w_in["wk"], 0, 2)
        ld(nc.sync, xt_tiles[0], xt_in, 0, 2, 0, 512)
        ld(nc.sync, w_sb["wk"], w_in["wk"], 2, EC)
        nc.sync.dma_start(out=cst32[:], in_=cst32_in[:, :])
        ld(nc.sync, xt_tiles[0], xt_in, 2, 4, 0, 512)
        ld(nc.sync, xt_tiles[0], xt_in, 4, EC, 0, 512)
        ld(nc.sync, w_sb["wq"], w_in["wq"], 0, EC)
        nc.sync.dma_start(out=cst16[:], in_=cst16_in[:, :])
        nc.sync.dma_start(out=qpos_sb[:, 0:512], in_=qpos_in[:, 0:512])
        for h in range(2):
            ld(nc.sync, xt_tiles[2], xt_in, h * 4, (h + 1) * 4, 1024, 1536)
        ld(nc.sync, w_sb["wv"], w_in["wv"], 0, EC)
        for h in range(2):
            ld(nc.sync, xt_tiles[1], xt_in, h * 4, (h + 1) * 4, 512, 1024)
        nc.sync.dma_start(out=qpos_sb[:, 512:2048], in_=qpos_in[:, 512:2048])
        for h in range(2):
            ld(nc.sync, xt_tiles[3], xt_in, h * 4, (h + 1) * 4, 1536, 2048)

        bq, bk = cst32[:, 0:1], cst32[:, 1:2]
        ones = cst16[:, 0:1]
        tpos = cst16[:, 1 : 1 + NK]

        kt_tiles = {}
        qt_tiles = {}
        v_big = consts.tile([P, NK, D], f16, name="v_big")

        def xcols(c, lo, hi):
            """xt slice for pos-cols [lo, hi) of chunk c."""
            qb, off = lo // 512, lo % 512
            assert hi <= (qb + 1) * 512
            return xt_tiles[qb][:, c, off : off + (hi - lo)]

        def proj_k_gen(g):
            pp = pj_psum.tile([P, 512], f32, tag="pj")
            for c in range(EC):
                nc.tensor.matmul(
                    pp[:], w_sb["wk"][:, c, :], xcols(c, g * 512, (g + 1) * 512),
                    start=(c == 0), stop=(c == EC - 1),
                )
                if c % 2 == 1:
                    yield
            kt = consts.tile([P, 512], f16, name=f"kt_{g}")
            nc.vector.tensor_scalar_add(kt[:], pp[:], bk)
            kt_tiles[g] = kt
            yield

        def proj_v_gen(g):
            for jp in range(2):
                vp = vv_psum.tile([P, 2, D], f32, tag="vv")
                for h in range(2):
                    jl = g * 4 + jp * 2 + h
                    for c in range(EC):
                        nc.tensor.matmul(
                            vp[:, h, :],
                            xcols(c, jl * P, (jl + 1) * P),
                            w_sb["wv"][:, c, :],
                            start=(c == 0), stop=(c == EC - 1),
                        )
                    yield
                nc.vector.tensor_copy(
                    v_big[:, g * 4 + jp * 2 : g * 4 + jp * 2 + 2, :], vp[:]
                )

        def proj_q_gen(s):
            # slot s queries: own pair cols [256s, 256s+256) and other pair
            # cols [1024 + 256s, +256)
            pp = pj_psum.tile([P, 512], f32, tag="pj")
            for half, base in ((0, 256 * s), (1, 1024 + 256 * s)):
                for c in range(EC):
                    nc.tensor.matmul(
                        pp[:, half * 256 : half * 256 + 256],
                        w_sb["wq"][:, c, :], xcols(c, base, base + 256),
                        start=(c == 0), stop=(c == EC - 1),
                    )
                    if c % 2 == 1:
                        yield
            qt = consts.tile([P, 512], f16, name=f"qt_{s}")
            nc.vector.tensor_scalar_add(qt[:], pp[:], bq)
            qt_tiles[s] = qt
            yield

        ot_sb = out_pool.tile([P, S], f32)
        rs_sb = out_pool.tile([1, S], f32)

        def attn_slot_gen(s):
            n_pr = s + 1          # key pairs 0..s; last pair masked
            qt = qt_tiles[s]
            qp = None
            ot = ot_psum.tile([P, 512], f32, tag="ot")
            rp = ot_psum.tile([1, 512], f32, tag="rs")
            racc = out_pool.tile([P, 512], f16, name=f"racc_{s}")

            def emit_pv(seq, pos, ptp):
                for h in (0, 1):
                    nc.tensor.matmul(
                        ot[:], v_big[:, 2 * pos + h, :], ptp[:, h, :],
                        start=(seq == 0 and h == 0),
                        stop=(seq == n_pr - 1 and h == 1),
                    )
                if n_pr > 1 and seq == n_pr - 2:
                    nc.tensor.matmul(rp[:], ones, racc[:], start=True, stop=False)

            pend = []
            # masked (diagonal) pair first: its longer exp->STT chain then
            # overlaps the plain pairs' exps instead of sitting in the tail
            order = [n_pr - 1] + list(range(n_pr - 1))
            for seq, pos in enumerate(order):
                j0 = 2 * pos
                stp = st_psum.tile([P, 2, 512], f32, tag="st")
                for h in (0, 1):
                    j = j0 + h
                    nc.tensor.matmul(
                        stp[:, h, :],
                        kt_tiles[j // 4][:, (j % 4) * P : (j % 4 + 1) * P],
                        qt[:], start=True, stop=True,
                    )
                ptp = pt_pool.tile([P, 2, 512], f16, tag="pt")
                masked = pos == n_pr - 1
                final = seq == n_pr - 1
                if masked:
                    # diagonal pair: split exp + per-half STT mask
                    for h in (0, 1):
                        j = j0 + h
                        nc.scalar.activation(
                            out=ptp[:, h, :], in_=stp[:, h, :], func=AF.Exp,
                            scale=SCALE,
                        )
                        nc.vector.scalar_tensor_tensor(
                            out=ptp[:, h, :],
                            in0=qpos_sb[:, s * 512 : (s + 1) * 512],
                            scalar=tpos[:, j : j + 1],
                            in1=ptp[:, h, :],
                            op0=mybir.AluOpType.is_ge,
                            op1=mybir.AluOpType.mult,
                        )
                elif final:
                    # split the final exp so its PV starts after half
                    for h in (0, 1):
                        nc.scalar.activation(
                            out=ptp[:, h, :], in_=stp[:, h, :], func=AF.Exp,
                            scale=SCALE,
                        )
                else:
                    nc.scalar.activation(
                        out=ptp[:], in_=stp[:], func=AF.Exp, scale=SCALE,
                    )
                if not final:
                    if seq == 0:
                        nc.vector.tensor_copy(racc[:], ptp[:, 0, :])
                        nc.vector.tensor_add(racc[:], racc[:], ptp[:, 1, :])
                    else:
                        nc.vector.tensor_add(racc[:], racc[:], ptp[:, 0, :])
                        nc.vector.tensor_add(racc[:], racc[:], ptp[:, 1, :])
                pend.append((seq, pos, ptp))
                yield
                if len(pend) > 2:
                    emit_pv(*pend.pop(0))
                yield
            while pend:
                seq, pos, ptp = pend.pop(0)
                if seq < n_pr - 1:
                    emit_pv(seq, pos, ptp)
                else:
                    for h in (0, 1):
                        nc.tensor.matmul(
                            ot[:], v_big[:, 2 * pos + h, :], ptp[:, h, :],
                            start=(seq == 0 and h == 0), stop=(h == 1),
                        )
                        nc.tensor.matmul(
                            rp[:], ones, ptp[:, h, :],
                            start=(n_pr == 1 and h == 0), stop=(h == 1),
                        )
            nc.vector.tensor_copy(ot_sb[:, s * 512 : s * 512 + 256], ot[:, 0:256])
            nc.scalar.copy(ot_sb[:, s * 512 + 256 : (s + 1) * 512], ot[:, 256:512])
            nc.vector.tensor_copy(rs_sb[0:1, s * 512 : (s + 1) * 512], rp[:])
            nc.sync.dma_start(
                out=rs_out[:, s * 512 : (s + 1) * 512],
                in_=rs_sb[0:1, s * 512 : (s + 1) * 512],
            )
            nc.sync.dma_start(
                out=ot_out[:, s * 512 : s * 512 + 256],
                in_=ot_sb[:, s * 512 : s * 512 + 256],
            )
            nc.scalar.dma_start(
                out=ot_out[:, s * 512 + 256 : (s + 1) * 512],
                in_=ot_sb[:, s * 512 + 256 : (s + 1) * 512],
            )

        def drain(gen):
            for _ in gen:
                pass

        def interleave(main, filler, ratio=1):
            for _ in main:
                for _ in range(ratio):
                    next(filler, None)

        def chain(*gens):
            for g in gens:
                yield from g

        # K for local tiles 0..3, Q0+Q1 up front, then the attention
        # cascade; later projections ride as fillers inside the windows so
        # each slot's qt is ready before its window opens.
        drain(proj_k_gen(0))
        drain(proj_q_gen(0))
        drain(proj_q_gen(1))
        f1 = chain(proj_v_gen(0), proj_q_gen(2))
        interleave(attn_slot_gen(0), f1, ratio=7)
        drain(f1)
        f2 = chain(proj_k_gen(1), proj_v_gen(1))
        interleave(attn_slot_gen(1), f2, ratio=3)
        drain(f2)
        f3 = proj_q_gen(3)
        interleave(attn_slot_gen(2), f3, ratio=2)
        drain(f3)
        drain(attn_slot_gen(3))

    nc.compile()
    return nc


_NC_CACHE = {}


def _get_nc():
    if "nc" not in _NC_CACHE:
        _NC_CACHE["nc"] = _build_nc()
    return _NC_CACHE["nc"]


def _get_runner():
    if "runner" in _NC_CACHE:
        return _NC_CACHE["runner"]

    import jax
    from jax.sharding import Mesh, PartitionSpec
    from jax.experimental.shard_map import shard_map
    from concourse import bass2jax, mybir

    nc = _get_nc()
    bass2jax.install_neuronx_cc_hook()

    partition_name = nc.partition_id_tensor.name if nc.partition_id_tensor else None
    in_names, out_names, out_avals = [], [], []
    for alloc in nc.m.functions[0].allocations:
        if not isinstance(alloc, mybir.MemoryLocationSet):
            continue
        name = alloc.memorylocations[0].name
        if alloc.kind == "ExternalInput":
            if name != partition_name:
                in_names.append(name)
        elif alloc.kind == "ExternalOutput":
            out_names.append(name)
            out_avals.append(
                jax.core.ShapedArray(tuple(alloc.tensor_shape), mybir.dt.np(alloc.dtype))
            )
    n_params = len(in_names)
    all_names = in_names + out_names
    if partition_name is not None:
        all_names = all_names + [partition_name]

    def _body(*args):
        operands = list(args)
        if partition_name is not None:
            operands.append(bass2jax.partition_id_tensor())
        outs = bass2jax._bass_exec_p.bind(
            *operands,
            out_avals=tuple(out_avals),
            in_names=tuple(all_names),
            out_names=tuple(out_names),
            lowering_input_output_aliases=(),
            sim_require_finite=True,
            sim_require_nnan=True,
            nc=nc,
        )
        return tuple(outs)

    devices = jax.devices()[:8]
    mesh = Mesh(np.asarray(devices), ("core",))
    sharded = jax.jit(
        shard_map(
            _body,
            mesh=mesh,
            in_specs=(PartitionSpec("core"),) * (n_params + len(out_names)),
            out_specs=(PartitionSpec("core"),) * len(out_names),
            check_rep=False,
        ),
        donate_argnums=tuple(range(n_params, n_params + len(out_names))),
        keep_unused=True,
    )
    runner = {
        "sharded": sharded,
        "in_names": in_names,
        "out_names": out_names,
        "out_avals": out_avals,
    }
    _NC_CACHE["runner"] = runner
    return runner


def _prep_in_concat(x, wq, bq, wk, bk, wv, bv):
    x = np.asarray(x, dtype=np.float32)

    if "pos2nat" not in _NC_CACHE:
        _NC_CACHE["pos2nat"] = [_pos_to_nat(0), _pos_to_nat(1)]
    p2n = _NC_CACHE["pos2nat"]

    def pack_w(w):
        return np.ascontiguousarray(
            np.asarray(w, np.float32).reshape(EC, P, D).transpose(1, 0, 2)
        ).astype(np.float16)

    w16 = {"wq": pack_w(wq), "wk": pack_w(wk), "wv": pack_w(wv)}
    _NC_CACHE["bv"] = np.asarray(bv, np.float32)

    cst32 = np.stack(
        [np.asarray(bq, np.float32), np.asarray(bk, np.float32)], axis=1
    )  # [P, 2]

    c16, qp = [], []
    for role in (0, 1):
        nat = p2n[role]
        t = np.empty((P, 1 + NK), np.float16)
        t[:, 0] = 1.0
        for j in range(NK):
            t[:, 1 + j] = (nat[j] * P + np.arange(P)).astype(np.float16)
        c16.append(t)
        colpos = (
            nat[:, None] * P + np.arange(P)[None, :]
        ).reshape(-1)                              # nat position of pos-col
        # slot order: slot s = own pair cols | other pair cols
        slotpos = np.concatenate(
            [np.concatenate([colpos[256 * s : 256 * s + 256],
                             colpos[1024 + 256 * s : 1024 + 256 * s + 256]])
             for s in range(4)]
        )
        qp.append(
            np.ascontiguousarray(
                np.tile(slotpos[None, :].astype(np.float16), (P, 1))
            )
        )
    _NC_CACHE["slotperm"] = []
    for r in (0, 1):
        colpos = (p2n[r][:, None] * P + np.arange(P)[None, :]).reshape(-1)
        _NC_CACHE["slotperm"].append(
            np.concatenate(
                [np.concatenate([colpos[256 * s : 256 * s + 256],
                                 colpos[1024 + 256 * s : 1024 + 256 * s + 256]])
                 for s in range(4)]
            )
        )
    _NC_CACHE["colperm"] = [
        (p2n[r][:, None] * P + np.arange(P)[None, :]).reshape(-1) for r in (0, 1)
    ]

    xt_cores = []
    for b in range(B):
        xbT = np.ascontiguousarray(x[b].T)  # [E, S]
        for role in (0, 1):
            xg = xbT[:, _NC_CACHE["colperm"][role]].astype(np.float16)
            xt_cores.append(
                np.ascontiguousarray(xg.reshape(EC, P, S).transpose(1, 0, 2))
            )

    runner = _get_runner()
    concat = {
        "xt": np.concatenate(xt_cores, axis=0),
        "cst32": np.concatenate([cst32] * 8, axis=0),
        "cst16": np.concatenate([c16[c % 2] for c in range(8)], axis=0),
        "qpos": np.concatenate([qp[c % 2] for c in range(8)], axis=0),
    }
    for n, v in w16.items():
        concat[n] = np.concatenate([v] * 8, axis=0)
    return [concat[n] for n in runner["in_names"]]


def _run_concat(concat_in):
    runner = _get_runner()
    zeros = [
        np.zeros((8 * a.shape[0], *a.shape[1:]), a.dtype) for a in runner["out_avals"]
    ]
    out_arrs = runner["sharded"](*concat_in, *zeros)
    ot = np.asarray(out_arrs[runner["out_names"].index("ot")]).reshape(8, P, S)
    rs = np.asarray(out_arrs[runner["out_names"].index("rs")]).reshape(8, S)
    return ot, rs


def _assemble(ot, rs):
    bv = _NC_CACHE["bv"]
    slotperm = _NC_CACHE["slotperm"]
    out = np.empty((B, S, D), dtype=np.float32)
    for b in range(B):
        acc_o = np.zeros((D, S), np.float64)
        acc_r = np.zeros((S,), np.float64)
        for role in (0, 1):
            c = 2 * b + role
            perm = slotperm[role]
            acc_o[:, perm] += ot[c]
            acc_r[perm] += rs[c]
        out[b] = (acc_o / acc_r[None, :]).T + bv[None, :]
    return out


def kernel(x, wq, bq, wk, bk, wv, bv):
    concat_in = _prep_in_concat(x, wq, bq, wk, bk, wv, bv)
    ot, rs = _run_concat(concat_in)
    return _assemble(ot, rs)


# revision 34
# speedup vs baseline: 1.0990x; 1.0416x over previous
"""Causal single-head attention (B=4, S=2048, E=1024, D=128) on 8 trn2 cores.

KEY-SPLIT sharding: 2 cores per batch. Core role r owns the key tiles with
nat index == r (mod 2) (8 of 16 128-row tiles) and computes K/V only for
those; BOTH cores process ALL 2048 queries against their own keys
(flash-style partials), and the host combines:
    out = (otA + otB) / (rsA + rsB)  (+bv, transpose)

xt column layout per core (pos space): cols 0..1023 = own key tiles
ascending, cols 1024..2047 = the other role's tiles ascending. Query
slot s = pos-tiles {own 2s, own 2s+1, other 2s, other 2s+1} (512 queries),
so slot s needs exactly the local key tiles 0..2s+1 for BOTH roles:
20 score units per core, no padding waste. Only the last key-tile pair of
each slot can straddle the diagonal; it is masked with a position-compare
STT (qpos >= tpos, host tables, role-baked).

Device program (fp16 operands, f32 PSUM, exp pairs over [128,1024]):
  K^T, V from own xt cols; Q^T per slot from (own pair | other pair) cols
  slot s, key pair u: st[t,q] pair -> exp pair -> (last pair: STT mask)
  racc (+)= pt halves [DVE];  ot[d,q] += v.T @ pt  [PE]
  rs = ones.T@racc + direct ones.T@pt for the last pair
"""

import math

import numpy as np

B, S, E, D = 4, 2048, 1024, 128
P = 128
EC = E // P          # 8 E-chunks
NK = 8               # own key tiles per core
SCALE = 1.0 / math.sqrt(D)


def _pos_to_nat(role):
    """pos-tile -> nat-tile mapping per role."""
    own = [2 * t + role for t in range(NK)]
    other = [2 * t + 1 - role for t in range(NK)]
    return np.array(own + other)


def _build_nc():
    from contextlib import ExitStack

    import concourse.bass as bass
    import concourse.tile as tile
    from concourse import bacc, mybir

    f16 = mybir.dt.float16
    f32 = mybir.dt.float32
    AF = mybir.ActivationFunctionType

    nc = bacc.Bacc("TRN2", target_bir_lowering=False, debug=False)

    xt_in = nc.dram_tensor("xt", [P, EC, S], f16, kind="ExternalInput")
    w_in = {
        n: nc.dram_tensor(n, [P, EC, D], f16, kind="ExternalInput")
        for n in ("wq", "wk", "wv")
    }
    # cst32: col0 = bq, col1 = bk
    cst32_in = nc.dram_tensor("cst32", [P, 2], f32, kind="ExternalInput")
    # ones column + tpos (own key nat positions per local tile)
    cst16_in = nc.dram_tensor("cst16", [P, 1 + NK], f16, kind="ExternalInput")
    # qpos: nat position of the query in each SLOT column (replicated rows)
    qpos_in = nc.dram_tensor("qpos", [P, S], f16, kind="ExternalInput")
    ot_out = nc.dram_tensor("ot", [P, S], f32, kind="ExternalOutput")
    rs_out = nc.dram_tensor("rs", [1, S], f32, kind="ExternalOutput")

    with tile.TileContext(nc) as tc, ExitStack() as ctx:
        consts = ctx.enter_context(tc.tile_pool(name="consts", bufs=1))
        xt_pool = ctx.enter_context(tc.tile_pool(name="xt", bufs=4))
        pt_pool = ctx.enter_context(tc.tile_pool(name="pt", bufs=6))
        out_pool = ctx.enter_context(tc.tile_pool(name="outp", bufs=1))
        pj_psum = ctx.enter_context(tc.tile_pool(name="pjp", bufs=2, space="PSUM"))
        vv_psum = ctx.enter_context(tc.tile_pool(name="vvp", bufs=1, space="PSUM"))
        st_psum = ctx.enter_context(tc.tile_pool(name="stp", bufs=3, space="PSUM"))
        ot_psum = ctx.enter_context(tc.tile_pool(name="otp", bufs=1, space="PSUM"))

        w_sb = {}
        for n in ("wk", "wv", "wq"):
            w_sb[n] = consts.tile([P, EC, D], f16, name=f"w_{n}")
        xt_tiles = {}   # 4 column-quarters of 512
        for qb in range(4):
            xt_tiles[qb] = xt_pool.tile([P, EC, 512], f16, tag="xt", name=f"xt_{qb}")
        cst32 = consts.tile([P, 2], f32)
        cst16 = consts.tile([P, 1 + NK], f16)
        qpos_sb = consts.tile([P, S], f16)

        def ld(q, sb, dram, ch0, ch1, col0=None, col1=None):
            if col0 is None:
                q.dma_start(out=sb[:, ch0:ch1], in_=dram[:, ch0:ch1])
            else:
                q.dma_start(out=sb[:, ch0:ch1, :], in_=dram[:, ch0:ch1, col0:col1])

        # priority order: wk + own K cols, then Q weights + other-block
        # cols + slot-0 tables (the attention cascade starts at slot 0)
        ld(nc.scalar, w_sb["wk"], w_in["wk"], 0, 2)
        ld(nc.sync, xt_tiles[0], xt_in, 0, 2, 0, 512)
        ld(nc.sync, w_sb["wk"], w_in["wk"], 2, EC)
        ld(nc.sync, xt_tiles[0], xt_in, 2, 4, 0, 512)
        ld(nc.sync, xt_tiles[0], xt_in, 4, EC, 0, 512)
        nc.sync.dma_start(out=cst32[:], in_=cst32_in[:, :])
        ld(nc.sync, w_sb["wq"], w_in["wq"], 0, EC)
        nc.sync.dma_start(out=cst16[:], in_=cst16_in[:, :])
        nc.sync.dma_start(out=qpos_sb[:, 0:512], in_=qpos_in[:, 0:512])
        for h in range(2):
            ld(nc.sync, xt_tiles[2], xt_in, h * 4, (h + 1) * 4, 1024, 1536)
        ld(nc.sync, w_sb["wv"], w_in["wv"], 0, EC)
        for h in range(2):
            ld(nc.sync, xt_tiles[1], xt_in, h * 4, (h + 1) * 4, 512, 1024)
        nc.sync.dma_start(out=qpos_sb[:, 512:2048], in_=qpos_in[:, 512:2048])
        for h in range(2):
            ld(nc.sync, xt_tiles[3], xt_in, h * 4, (h + 1) * 4, 1536, 2048)

        bq, bk = cst32[:, 0:1], cst32[:, 1:2]
        ones = cst16[:, 0:1]
        tpos = cst16[:, 1 : 1 + NK]

        kt_tiles = {}
        qt_tiles = {}
        v_big = consts.tile([P, NK, D], f16, name="v_big")

        def xcols(c, lo, hi):
            """xt slice for pos-cols [lo, hi) of chunk c."""
            qb, off = lo // 512, lo % 512
            assert hi <= (qb + 1) * 512
            return xt_tiles[qb][:, c, off : off + (hi - lo)]

        def proj_k_gen(g):
            pp = pj_psum.tile([P, 512], f32, tag="pj")
            for c in range(EC):
                nc.tensor.matmul(
                    pp[:], w_sb["wk"][:, c, :], xcols(c, g * 512, (g + 1) * 512),
                    start=(c == 0), stop=(c == EC - 1),
                )
                if c % 2 == 1:
                    yield
            kt = consts.tile([P, 512], f16, name=f"kt_{g}")
            nc.vector.tensor_scalar_add(kt[:], pp[:], bk)
            kt_tiles[g] = kt
            yield

        def proj_v_gen(g):
            for jp in range(2):
                vp = vv_psum.tile([P, 2, D], f32, tag="vv")
                for h in range(2):
                    jl = g * 4 + jp * 2 + h
                    for c in range(EC):
                        nc.tensor.matmul(
                            vp[:, h, :],
                            xcols(c, jl * P, (jl + 1) * P),
                            w_sb["wv"][:, c, :],
                            start=(c == 0), stop=(c == EC - 1),
                        )
                    yield
                nc.vector.tensor_copy(
                    v_big[:, g * 4 + jp * 2 : g * 4 + jp * 2 + 2, :], vp[:]
                )

        def proj_q_gen(s):
            # slot s queries: own pair cols [256s, 256s+256) and other pair
            # cols [1024 + 256s, +256)
            pp = pj_psum.tile([P, 512], f32, tag="pj")
            for half, base in ((0, 256 * s), (1, 1024 + 256 * s)):
                for c in range(EC):
                    nc.tensor.matmul(
                        pp[:, half * 256 : half * 256 + 256],
                        w_sb["wq"][:, c, :], xcols(c, base, base + 256),
                        start=(c == 0), stop=(c == EC - 1),
                    )
                    if c % 2 == 1:
                        yield
            qt = consts.tile([P, 512], f16, name=f"qt_{s}")
            nc.vector.tensor_scalar_add(qt[:], pp[:], bq)
            qt_tiles[s] = qt
            yield

        ot_sb = out_pool.tile([P, S], f32)
        rs_sb = out_pool.tile([1, S], f32)

        def attn_slot_gen(s):
            n_pr = s + 1          # key pairs 0..s; last pair masked
            qt = qt_tiles[s]
            qp = None
            ot = ot_psum.tile([P, 512], f32, tag="ot")
            rp = ot_psum.tile([1, 512], f32, tag="rs")
            racc = out_pool.tile([P, 512], f16, name=f"racc_{s}")

            tail_trick = True        # direct-rs drain for every slot

            def emit_pv(seq, pos, ptp):
                for h in (0, 1):
                    nc.tensor.matmul(
                        ot[:], v_big[:, 2 * pos + h, :], ptp[:, h, :],
                        start=(seq == 0 and h == 0),
                        stop=(seq == n_pr - 1 and h == 1),
                    )
                if tail_trick and n_pr > 1 and seq == n_pr - 2:
                    nc.tensor.matmul(rp[:], ones, racc[:], start=True, stop=False)

            pend = []
            # masked (diagonal) pair first: its longer exp->STT chain then
            # overlaps the plain pairs' exps instead of sitting in the tail
            order = [n_pr - 1] + list(range(n_pr - 1))
            for seq, pos in enumerate(order):
                j0 = 2 * pos
                ptp = pt_pool.tile([P, 2, 512], f16, tag="pt")
                masked = pos == n_pr - 1
                final = seq == n_pr - 1
                for h in (0, 1):
                    j = j0 + h
                    sth = st_psum.tile([P, 512], f32, tag="st")
                    nc.tensor.matmul(
                        sth[:],
                        kt_tiles[j // 4][:, (j % 4) * P : (j % 4 + 1) * P],
                        qt[:], start=True, stop=True,
                    )
                    nc.scalar.activation(
                        out=ptp[:, h, :], in_=sth[:], func=AF.Exp, scale=SCALE,
                    )
                    if masked:
                        nc.vector.scalar_tensor_tensor(
                            out=ptp[:, h, :],
                            in0=qpos_sb[:, s * 512 : (s + 1) * 512],
                            scalar=tpos[:, j : j + 1],
                            in1=ptp[:, h, :],
                            op0=mybir.AluOpType.is_ge,
                            op1=mybir.AluOpType.mult,
                        )
                if not (final and tail_trick):
                    if seq == 0:
                        nc.vector.tensor_copy(racc[:], ptp[:, 0, :])
                        nc.vector.tensor_add(racc[:], racc[:], ptp[:, 1, :])
                    else:
                        nc.vector.tensor_add(racc[:], racc[:], ptp[:, 0, :])
                        nc.vector.tensor_add(racc[:], racc[:], ptp[:, 1, :])
                pend.append((seq, pos, ptp))
                yield
                if len(pend) > 2:
                    emit_pv(*pend.pop(0))
                yield
            while pend:
                seq, pos, ptp = pend.pop(0)
                if seq < n_pr - 1 or not tail_trick:
                    emit_pv(seq, pos, ptp)
                    if not tail_trick and seq == n_pr - 1:
                        nc.tensor.matmul(rp[:], ones, racc[:], start=True, stop=True)
                else:
                    for h in (0, 1):
                        nc.tensor.matmul(
                            ot[:], v_big[:, 2 * pos + h, :], ptp[:, h, :],
                            start=(seq == 0 and h == 0), stop=(h == 1),
                        )
                        nc.tensor.matmul(
                            rp[:], ones, ptp[:, h, :],
                            start=(n_pr == 1 and h == 0), stop=(h == 1),
                        )
            nc.vector.tensor_copy(ot_sb[:, s * 512 : s * 512 + 256], ot[:, 0:256])
            nc.scalar.copy(ot_sb[:, s * 512 + 256 : (s + 1) * 512], ot[:, 256:512])
            nc.vector.tensor_copy(rs_sb[0:1, s * 512 : (s + 1) * 512], rp[:])
            nc.sync.dma_start(
                out=rs_out[:, s * 512 : (s + 1) * 512],
                in_=rs_sb[0:1, s * 512 : (s + 1) * 512],
            )
            nc.sync.dma_start(
                out=ot_out[:, s * 512 : s * 512 + 256],
                in_=ot_sb[:, s * 512 : s * 512 + 256],
            )
            nc.scalar.dma_start(
                out=ot_out[:, s * 512 + 256 : (s + 1) * 512],
                in_=ot_sb[:, s * 512 + 256 : (s + 1) * 512],
            )

        def drain(gen):
            for _ in gen:
                pass

        def interleave(main, filler, ratio=1):
            for _ in main:
                for _ in range(ratio):
                    next(filler, None)

        def chain(*gens):
            for g in gens:
                yield from g

        # K for local tiles 0..3, Q0+Q1 up front, then the attention
        # cascade; later projections ride as fillers inside the windows so
        # each slot's qt is ready before its window opens.
        drain(proj_k_gen(0))
        drain(proj_q_gen(0))
        drain(proj_q_gen(1))
        f1 = chain(proj_v_gen(0), proj_q_gen(2))
        interleave(attn_slot_gen(0), f1, ratio=7)
        drain(f1)
        f2 = chain(proj_k_gen(1), proj_v_gen(1))
        interleave(attn_slot_gen(1), f2, ratio=3)
        drain(f2)
        f3 = proj_q_gen(3)
        interleave(attn_slot_gen(2), f3, ratio=2)
        drain(f3)
        drain(attn_slot_gen(3))

    nc.compile()
    return nc


_NC_CACHE = {}


def _get_nc():
    if "nc" not in _NC_CACHE:
        _NC_CACHE["nc"] = _build_nc()
    return _NC_CACHE["nc"]


def _get_runner():
    if "runner" in _NC_CACHE:
        return _NC_CACHE["runner"]

    import jax
    from jax.sharding import Mesh, PartitionSpec
    from jax.experimental.shard_map import shard_map
    from concourse import bass2jax, mybir

    nc = _get_nc()
    bass2jax.install_neuronx_cc_hook()

    partition_name = nc.partition_id_tensor.name if nc.partition_id_tensor else None
    in_names, out_names, out_avals = [], [], []
    for alloc in nc.m.functions[0].allocations:
        if not isinstance(alloc, mybir.MemoryLocationSet):
            continue
        name = alloc.memorylocations[0].name
        if alloc.kind == "ExternalInput":
            if name != partition_name:
                in_names.append(name)
        elif alloc.kind == "ExternalOutput":
            out_names.append(name)
            out_avals.append(
                jax.core.ShapedArray(tuple(alloc.tensor_shape), mybir.dt.np(alloc.dtype))
            )
    n_params = len(in_names)
    all_names = in_names + out_names
    if partition_name is not None:
        all_names = all_names + [partition_name]

    def _body(*args):
        operands = list(args)
        if partition_name is not None:
            operands.append(bass2jax.partition_id_tensor())
        outs = bass2jax._bass_exec_p.bind(
            *operands,
            out_avals=tuple(out_avals),
            in_names=tuple(all_names),
            out_names=tuple(out_names),
            lowering_input_output_aliases=(),
            sim_require_finite=True,
            sim_require_nnan=True,
            nc=nc,
        )
        return tuple(outs)

    devices = jax.devices()[:8]
    mesh = Mesh(np.asarray(devices), ("core",))
    sharded = jax.jit(
        shard_map(
            _body,
            mesh=mesh,
            in_specs=(PartitionSpec("core"),) * (n_params + len(out_names)),
            out_specs=(PartitionSpec("core"),) * len(out_names),
            check_rep=False,
        ),
        donate_argnums=tuple(range(n_params, n_params + len(out_names))),
        keep_unused=True,
    )
    runner = {
        "sharded": sharded,
        "in_names": in_names,
        "out_names": out_names,
        "out_avals": out_avals,
    }
    _NC_CACHE["runner"] = runner
    return runner


def _prep_in_concat(x, wq, bq, wk, bk, wv, bv):
    x = np.asarray(x, dtype=np.float32)

    if "pos2nat" not in _NC_CACHE:
        _NC_CACHE["pos2nat"] = [_pos_to_nat(0), _pos_to_nat(1)]
    p2n = _NC_CACHE["pos2nat"]

    def pack_w(w):
        return np.ascontiguousarray(
            np.asarray(w, np.float32).reshape(EC, P, D).transpose(1, 0, 2)
        ).astype(np.float16)

    w16 = {"wq": pack_w(wq), "wk": pack_w(wk), "wv": pack_w(wv)}
    _NC_CACHE["bv"] = np.asarray(bv, np.float32)

    cst32 = np.stack(
        [np.asarray(bq, np.float32), np.asarray(bk, np.float32)], axis=1
    )  # [P, 2]

    c16, qp = [], []
    for role in (0, 1):
        nat = p2n[role]
        t = np.empty((P, 1 + NK), np.float16)
        t[:, 0] = 1.0
        for j in range(NK):
            t[:, 1 + j] = (nat[j] * P + np.arange(P)).astype(np.float16)
        c16.append(t)
        colpos = (
            nat[:, None] * P + np.arange(P)[None, :]
        ).reshape(-1)                              # nat position of pos-col
        # slot order: slot s = own pair cols | other pair cols
        slotpos = np.concatenate(
            [np.concatenate([colpos[256 * s : 256 * s + 256],
                             colpos[1024 + 256 * s : 1024 + 256 * s + 256]])
             for s in range(4)]
        )
        qp.append(
            np.ascontiguousarray(
                np.tile(slotpos[None, :].astype(np.float16), (P, 1))
            )
        )
    _NC_CACHE["slotperm"] = []
    for r in (0, 1):
        colpos = (p2n[r][:, None] * P + np.arange(P)[None, :]).reshape(-1)
        _NC_CACHE["slotperm"].append(
            np.concatenate(
                [np.concatenate([colpos[256 * s : 256 * s + 256],
                                 colpos[1024 + 256 * s : 1024 + 256 * s + 256]])
                 for s in range(4)]
            )
        )
    _NC_CACHE["colperm"] = [
        (p2n[r][:, None] * P + np.arange(P)[None, :]).reshape(-1) for r in (0, 1)
    ]

    xt_cores = []
    for b in range(B):
        xbT = np.ascontiguousarray(x[b].T)  # [E, S]
        for role in (0, 1):
            xg = xbT[:, _NC_CACHE["colperm"][role]].astype(np.float16)
            xt_cores.append(
                np.ascontiguousarray(xg.reshape(EC, P, S).transpose(1, 0, 2))
            )

    runner = _get_runner()
    concat = {
        "xt": np.concatenate(xt_cores, axis=0),
        "cst32": np.concatenate([cst32] * 8, axis=0),
        "cst16": np.concatenate([c16[c % 2] for c in range(8)], axis=0),
        "qpos": np.concatenate([qp[c % 2] for c in range(8)], axis=0),
    }
    for n, v in w16.items():
        concat[n] = np.concatenate([v] * 8, axis=0)
    return [concat[n] for n in runner["in_names"]]


def _run_concat(concat_in):
    runner = _get_runner()
    zeros = [
        np.zeros((8 * a.shape[0], *a.shape[1:]), a.dtype) for a in runner["out_avals"]
    ]
    out_arrs = runner["sharded"](*concat_in, *zeros)
    ot = np.asarray(out_arrs[runner["out_names"].index("ot")]).reshape(8, P, S)
    rs = np.asarray(out_arrs[runner["out_names"].index("rs")]).reshape(8, S)
    return ot, rs


def _assemble(ot, rs):
    bv = _NC_CACHE["bv"]
    slotperm = _NC_CACHE["slotperm"]
    out = np.empty((B, S, D), dtype=np.float32)
    for b in range(B):
        acc_o = np.zeros((D, S), np.float64)
        acc_r = np.zeros((S,), np.float64)
        for role in (0, 1):
            c = 2 * b + role
            perm = slotperm[role]
            acc_o[:, perm] += ot[c]
            acc_r[perm] += rs[c]
        out[b] = (acc_o / acc_r[None, :]).T + bv[None, :]
    return out


def kernel(x, wq, bq, wk, bk, wv, bv):
    concat_in = _prep_in_concat(x, wq, bq, wk, bk, wv, bv)
    ot, rs = _run_concat(concat_in)
    return _assemble(ot, rs)


# revision 35
# speedup vs baseline: 1.1002x; 1.0011x over previous
"""Causal single-head attention (B=4, S=2048, E=1024, D=128) on 8 trn2 cores.

KEY-SPLIT sharding: 2 cores per batch. Core role r owns the key tiles with
nat index == r (mod 2) (8 of 16 128-row tiles) and computes K/V only for
those; BOTH cores process ALL 2048 queries against their own keys
(flash-style partials), and the host combines:
    out = (otA + otB) / (rsA + rsB)  (+bv, transpose)

xt column layout per core (pos space): cols 0..1023 = own key tiles
ascending, cols 1024..2047 = the other role's tiles ascending. Query
slot s = pos-tiles {own 2s, own 2s+1, other 2s, other 2s+1} (512 queries),
so slot s needs exactly the local key tiles 0..2s+1 for BOTH roles:
20 score units per core, no padding waste. Only the last key-tile pair of
each slot can straddle the diagonal; it is masked with a position-compare
STT (qpos >= tpos, host tables, role-baked).

Device program (fp16 operands, f32 PSUM, exp pairs over [128,1024]):
  K^T, V from own xt cols; Q^T per slot from (own pair | other pair) cols
  slot s, key pair u: st[t,q] pair -> exp pair -> (last pair: STT mask)
  racc (+)= pt halves [DVE];  ot[d,q] += v.T @ pt  [PE]
  rs = ones.T@racc + direct ones.T@pt for the last pair
"""

import math

import numpy as np

B, S, E, D = 4, 2048, 1024, 128
P = 128
EC = E // P          # 8 E-chunks
NK = 8               # own key tiles per core
SCALE = 1.0 / math.sqrt(D)


def _pos_to_nat(role):
    """pos-tile -> nat-tile mapping per role."""
    own = [2 * t + role for t in range(NK)]
    other = [2 * t + 1 - role for t in range(NK)]
    return np.array(own + other)


def _build_nc():
    from contextlib import ExitStack

    import concourse.bass as bass
    import concourse.tile as tile
    from concourse import bacc, mybir

    f16 = mybir.dt.float16
    f32 = mybir.dt.float32
    AF = mybir.ActivationFunctionType

    nc = bacc.Bacc("TRN2", target_bir_lowering=False, debug=False)

    xt_in = nc.dram_tensor("xt", [P, EC, S], f16, kind="ExternalInput")
    w_in = {
        n: nc.dram_tensor(n, [P, EC, D], f16, kind="ExternalInput")
        for n in ("wq", "wk", "wv")
    }
    # cst32: col0 = bq, col1 = bk
    cst32_in = nc.dram_tensor("cst32", [P, 2], f32, kind="ExternalInput")
    # ones column + tpos (own key nat positions per local tile)
    cst16_in = nc.dram_tensor("cst16", [P, 1 + NK], f16, kind="ExternalInput")
    # qpos: nat position of the query in each SLOT column (replicated rows)
    qpos_in = nc.dram_tensor("qpos", [P, S], f16, kind="ExternalInput")
    ot_out = nc.dram_tensor("ot", [P, S], f32, kind="ExternalOutput")
    rs_out = nc.dram_tensor("rs", [1, S], f32, kind="ExternalOutput")

    with tile.TileContext(nc) as tc, ExitStack() as ctx:
        consts = ctx.enter_context(tc.tile_pool(name="consts", bufs=1))
        xt_pool = ctx.enter_context(tc.tile_pool(name="xt", bufs=4))
        pt_pool = ctx.enter_context(tc.tile_pool(name="pt", bufs=8))
        out_pool = ctx.enter_context(tc.tile_pool(name="outp", bufs=1))
        pj_psum = ctx.enter_context(tc.tile_pool(name="pjp", bufs=2, space="PSUM"))
        vv_psum = ctx.enter_context(tc.tile_pool(name="vvp", bufs=1, space="PSUM"))
        st_psum = ctx.enter_context(tc.tile_pool(name="stp", bufs=3, space="PSUM"))
        ot_psum = ctx.enter_context(tc.tile_pool(name="otp", bufs=1, space="PSUM"))

        w_sb = {}
        for n in ("wk", "wv", "wq"):
            w_sb[n] = consts.tile([P, EC, D], f16, name=f"w_{n}")
        xt_tiles = {}   # 4 column-quarters of 512
        for qb in range(4):
            xt_tiles[qb] = xt_pool.tile([P, EC, 512], f16, tag="xt", name=f"xt_{qb}")
        cst32 = consts.tile([P, 2], f32)
        cst16 = consts.tile([P, 1 + NK], f16)
        qpos_sb = consts.tile([P, S], f16)

        def ld(q, sb, dram, ch0, ch1, col0=None, col1=None):
            if col0 is None:
                q.dma_start(out=sb[:, ch0:ch1], in_=dram[:, ch0:ch1])
            else:
                q.dma_start(out=sb[:, ch0:ch1, :], in_=dram[:, ch0:ch1, col0:col1])

        # priority: the critical chain is xt2 -> Q0 -> slot0, and HWDGE
        # costs ~625ns per DMA on one queue — so merge the xt loads into
        # whole-tile DMAs and split the early ones across both queues.
        ld(nc.scalar, w_sb["wk"], w_in["wk"], 0, 2)
        ld(nc.sync, xt_tiles[0], xt_in, 0, 2, 0, 512)
        ld(nc.sync, w_sb["wk"], w_in["wk"], 2, EC)
        ld(nc.sync, xt_tiles[0], xt_in, 2, 4, 0, 512)
        ld(nc.sync, xt_tiles[0], xt_in, 4, EC, 0, 512)
        nc.sync.dma_start(out=cst32[:], in_=cst32_in[:, :])
        ld(nc.sync, w_sb["wq"], w_in["wq"], 0, EC)
        nc.sync.dma_start(out=cst16[:], in_=cst16_in[:, :])
        nc.sync.dma_start(out=qpos_sb[:, 0:512], in_=qpos_in[:, 0:512])
        for h in range(4):
            ld(nc.sync, xt_tiles[2], xt_in, h * 2, (h + 1) * 2, 1024, 1536)
        ld(nc.sync, w_sb["wv"], w_in["wv"], 0, EC)
        for h in range(2):
            ld(nc.sync, xt_tiles[1], xt_in, h * 4, (h + 1) * 4, 512, 1024)
        nc.sync.dma_start(out=qpos_sb[:, 512:2048], in_=qpos_in[:, 512:2048])
        for h in range(2):
            ld(nc.sync, xt_tiles[3], xt_in, h * 4, (h + 1) * 4, 1536, 2048)

        bq, bk = cst32[:, 0:1], cst32[:, 1:2]
        ones = cst16[:, 0:1]
        tpos = cst16[:, 1 : 1 + NK]

        kt_tiles = {}
        qt_tiles = {}
        v_big = consts.tile([P, NK, D], f16, name="v_big")

        def xcols(c, lo, hi):
            """xt slice for pos-cols [lo, hi) of chunk c."""
            qb, off = lo // 512, lo % 512
            assert hi <= (qb + 1) * 512
            return xt_tiles[qb][:, c, off : off + (hi - lo)]

        def proj_k_gen(g):
            # `warm` dummy matmuls on already-resident data are interleaved
            # between the chunk pairs: during the DMA-feed-limited start the
            # PE would otherwise idle >1us between chunks, resetting the
            # p-state ramp (matmuls then run at 0.65GHz instead of 2.4GHz).
            pp = pj_psum.tile([P, 512], f32, tag="pj")
            for c in range(EC):
                nc.tensor.matmul(
                    pp[:], w_sb["wk"][:, c, :], xcols(c, g * 512, (g + 1) * 512),
                    start=(c == 0), stop=(c == EC - 1),
                )
                if c % 2 == 1:
                    yield
            kt = consts.tile([P, 512], f16, name=f"kt_{g}")
            nc.vector.tensor_scalar_add(kt[:], pp[:], bk)
            kt_tiles[g] = kt
            yield

        def proj_v_gen(g):
            for jp in range(2):
                vp = vv_psum.tile([P, 2, D], f32, tag="vv")
                for h in range(2):
                    jl = g * 4 + jp * 2 + h
                    for c in range(EC):
                        nc.tensor.matmul(
                            vp[:, h, :],
                            xcols(c, jl * P, (jl + 1) * P),
                            w_sb["wv"][:, c, :],
                            start=(c == 0), stop=(c == EC - 1),
                        )
                    yield
                nc.vector.tensor_copy(
                    v_big[:, g * 4 + jp * 2 : g * 4 + jp * 2 + 2, :], vp[:]
                )

        def proj_q_gen(s):
            # slot s queries: own pair cols [256s, 256s+256) and other pair
            # cols [1024 + 256s, +256)
            pp = pj_psum.tile([P, 512], f32, tag="pj")
            for half, base in ((0, 256 * s), (1, 1024 + 256 * s)):
                for c in range(EC):
                    nc.tensor.matmul(
                        pp[:, half * 256 : half * 256 + 256],
                        w_sb["wq"][:, c, :], xcols(c, base, base + 256),
                        start=(c == 0), stop=(c == EC - 1),
                    )
                    if c % 2 == 1:
                        yield
            qt = consts.tile([P, 512], f16, name=f"qt_{s}")
            nc.vector.tensor_scalar_add(qt[:], pp[:], bq)
            qt_tiles[s] = qt
            yield

        ot_sb = out_pool.tile([P, S], f32)
        rs_sb = out_pool.tile([1, S], f32)

        def attn_slot_gen(s):
            n_pr = s + 1          # key pairs 0..s; last pair masked
            qt = qt_tiles[s]
            qp = None
            ot = ot_psum.tile([P, 512], f32, tag="ot")
            rp = ot_psum.tile([1, 512], f32, tag="rs")
            racc = out_pool.tile([P, 512], f16, name=f"racc_{s}")

            tail_trick = True        # direct-rs drain for every slot

            def emit_pv(seq, pos, ptp):
                for h in (0, 1):
                    nc.tensor.matmul(
                        ot[:], v_big[:, 2 * pos + h, :], ptp[:, h, :],
                        start=(seq == 0 and h == 0),
                        stop=(seq == n_pr - 1 and h == 1),
                    )
                if tail_trick and n_pr > 1 and seq == n_pr - 2:
                    nc.tensor.matmul(rp[:], ones, racc[:], start=True, stop=False)

            pend = []
            # masked (diagonal) pair first: its longer exp->STT chain then
            # overlaps the plain pairs' exps instead of sitting in the tail
            order = [n_pr - 1] + list(range(n_pr - 1))
            for seq, pos in enumerate(order):
                j0 = 2 * pos
                ptp = pt_pool.tile([P, 2, 512], f16, tag="pt")
                masked = pos == n_pr - 1
                final = seq == n_pr - 1
                for h in (0, 1):
                    j = j0 + h
                    sth = st_psum.tile([P, 512], f32, tag="st")
                    nc.tensor.matmul(
                        sth[:],
                        kt_tiles[j // 4][:, (j % 4) * P : (j % 4 + 1) * P],
                        qt[:], start=True, stop=True,
                    )
                    nc.scalar.activation(
                        out=ptp[:, h, :], in_=sth[:], func=AF.Exp, scale=SCALE,
                    )
                    if masked:
                        nc.vector.scalar_tensor_tensor(
                            out=ptp[:, h, :],
                            in0=qpos_sb[:, s * 512 : (s + 1) * 512],
                            scalar=tpos[:, j : j + 1],
                            in1=ptp[:, h, :],
                            op0=mybir.AluOpType.is_ge,
                            op1=mybir.AluOpType.mult,
                        )
                if not (final and tail_trick):
                    if seq == 0:
                        nc.vector.tensor_copy(racc[:], ptp[:, 0, :])
                        nc.vector.tensor_add(racc[:], racc[:], ptp[:, 1, :])
                    else:
                        nc.vector.tensor_add(racc[:], racc[:], ptp[:, 0, :])
                        nc.vector.tensor_add(racc[:], racc[:], ptp[:, 1, :])
                pend.append((seq, pos, ptp))
                yield
                if len(pend) > 2:
                    emit_pv(*pend.pop(0))
                yield
            while pend:
                seq, pos, ptp = pend.pop(0)
                if seq < n_pr - 1 or not tail_trick:
                    emit_pv(seq, pos, ptp)
                    if not tail_trick and seq == n_pr - 1:
                        nc.tensor.matmul(rp[:], ones, racc[:], start=True, stop=True)
                else:
                    for h in (0, 1):
                        nc.tensor.matmul(
                            ot[:], v_big[:, 2 * pos + h, :], ptp[:, h, :],
                            start=(seq == 0 and h == 0), stop=(h == 1),
                        )
                        nc.tensor.matmul(
                            rp[:], ones, ptp[:, h, :],
                            start=(n_pr == 1 and h == 0), stop=(h == 1),
                        )
            nc.vector.tensor_copy(ot_sb[:, s * 512 : s * 512 + 256], ot[:, 0:256])
            nc.scalar.copy(ot_sb[:, s * 512 + 256 : (s + 1) * 512], ot[:, 256:512])
            nc.vector.tensor_copy(rs_sb[0:1, s * 512 : (s + 1) * 512], rp[:])
            nc.sync.dma_start(
                out=rs_out[:, s * 512 : (s + 1) * 512],
                in_=rs_sb[0:1, s * 512 : (s + 1) * 512],
            )
            nc.sync.dma_start(
                out=ot_out[:, s * 512 : s * 512 + 256],
                in_=ot_sb[:, s * 512 : s * 512 + 256],
            )
            nc.scalar.dma_start(
                out=ot_out[:, s * 512 + 256 : (s + 1) * 512],
                in_=ot_sb[:, s * 512 + 256 : (s + 1) * 512],
            )

        def drain(gen):
            for _ in gen:
                pass

        def interleave(main, filler, ratio=1):
            for _ in main:
                for _ in range(ratio):
                    next(filler, None)

        def chain(*gens):
            for g in gens:
                yield from g

        # K for local tiles 0..3, Q0+Q1 up front, then the attention
        # cascade; later projections ride as fillers inside the windows so
        # each slot's qt is ready before its window opens.
        drain(proj_k_gen(0))
        drain(proj_q_gen(0))
        f1 = chain(proj_q_gen(1), proj_v_gen(0), proj_q_gen(2))
        interleave(attn_slot_gen(0), f1, ratio=7)
        drain(f1)
        f2 = chain(proj_k_gen(1), proj_v_gen(1))
        interleave(attn_slot_gen(1), f2, ratio=3)
        drain(f2)
        f3 = proj_q_gen(3)
        interleave(attn_slot_gen(2), f3, ratio=2)
        drain(f3)
        drain(attn_slot_gen(3))

    nc.compile()
    return nc


_NC_CACHE = {}


def _get_nc():
    if "nc" not in _NC_CACHE:
        _NC_CACHE["nc"] = _build_nc()
    return _NC_CACHE["nc"]


def _get_runner():
    if "runner" in _NC_CACHE:
        return _NC_CACHE["runner"]

    import jax
    from jax.sharding import Mesh, PartitionSpec
    from jax.experimental.shard_map import shard_map
    from concourse import bass2jax, mybir

    nc = _get_nc()
    bass2jax.install_neuronx_cc_hook()

    partition_name = nc.partition_id_tensor.name if nc.partition_id_tensor else None
    in_names, out_names, out_avals = [], [], []
    for alloc in nc.m.functions[0].allocations:
        if not isinstance(alloc, mybir.MemoryLocationSet):
            continue
        name = alloc.memorylocations[0].name
        if alloc.kind == "ExternalInput":
            if name != partition_name:
                in_names.append(name)
        elif alloc.kind == "ExternalOutput":
            out_names.append(name)
            out_avals.append(
                jax.core.ShapedArray(tuple(alloc.tensor_shape), mybir.dt.np(alloc.dtype))
            )
    n_params = len(in_names)
    all_names = in_names + out_names
    if partition_name is not None:
        all_names = all_names + [partition_name]

    def _body(*args):
        operands = list(args)
        if partition_name is not None:
            operands.append(bass2jax.partition_id_tensor())
        outs = bass2jax._bass_exec_p.bind(
            *operands,
            out_avals=tuple(out_avals),
            in_names=tuple(all_names),
            out_names=tuple(out_names),
            lowering_input_output_aliases=(),
            sim_require_finite=True,
            sim_require_nnan=True,
            nc=nc,
        )
        return tuple(outs)

    devices = jax.devices()[:8]
    mesh = Mesh(np.asarray(devices), ("core",))
    sharded = jax.jit(
        shard_map(
            _body,
            mesh=mesh,
            in_specs=(PartitionSpec("core"),) * (n_params + len(out_names)),
            out_specs=(PartitionSpec("core"),) * len(out_names),
            check_rep=False,
        ),
        donate_argnums=tuple(range(n_params, n_params + len(out_names))),
        keep_unused=True,
    )
    runner = {
        "sharded": sharded,
        "in_names": in_names,
        "out_names": out_names,
        "out_avals": out_avals,
    }
    _NC_CACHE["runner"] = runner
    return runner


def _prep_in_concat(x, wq, bq, wk, bk, wv, bv):
    x = np.asarray(x, dtype=np.float32)

    if "pos2nat" not in _NC_CACHE:
        _NC_CACHE["pos2nat"] = [_pos_to_nat(0), _pos_to_nat(1)]
    p2n = _NC_CACHE["pos2nat"]

    def pack_w(w):
        return np.ascontiguousarray(
            np.asarray(w, np.float32).reshape(EC, P, D).transpose(1, 0, 2)
        ).astype(np.float16)

    w16 = {"wq": pack_w(wq), "wk": pack_w(wk), "wv": pack_w(wv)}
    _NC_CACHE["bv"] = np.asarray(bv, np.float32)

    cst32 = np.stack(
        [np.asarray(bq, np.float32), np.asarray(bk, np.float32)], axis=1
    )  # [P, 2]

    c16, qp = [], []
    for role in (0, 1):
        nat = p2n[role]
        t = np.empty((P, 1 + NK), np.float16)
        t[:, 0] = 1.0
        for j in range(NK):
            t[:, 1 + j] = (nat[j] * P + np.arange(P)).astype(np.float16)
        c16.append(t)
        colpos = (
            nat[:, None] * P + np.arange(P)[None, :]
        ).reshape(-1)                              # nat position of pos-col
        # slot order: slot s = own pair cols | other pair cols
        slotpos = np.concatenate(
            [np.concatenate([colpos[256 * s : 256 * s + 256],
                             colpos[1024 + 256 * s : 1024 + 256 * s + 256]])
             for s in range(4)]
        )
        qp.append(
            np.ascontiguousarray(
                np.tile(slotpos[None, :].astype(np.float16), (P, 1))
            )
        )
    _NC_CACHE["slotperm"] = []
    for r in (0, 1):
        colpos = (p2n[r][:, None] * P + np.arange(P)[None, :]).reshape(-1)
        _NC_CACHE["slotperm"].append(
            np.concatenate(
                [np.concatenate([colpos[256 * s : 256 * s + 256],
                                 colpos[1024 + 256 * s : 1024 + 256 * s + 256]])
                 for s in range(4)]
            )
        )
    _NC_CACHE["colperm"] = [
        (p2n[r][:, None] * P + np.arange(P)[None, :]).reshape(-1) for r in (0, 1)
    ]

    xt_cores = []
    for b in range(B):
        xbT = np.ascontiguousarray(x[b].T)  # [E, S]
        for role in (0, 1):
            xg = xbT[:, _NC_CACHE["colperm"][role]].astype(np.float16)
            xt_cores.append(
                np.ascontiguousarray(xg.reshape(EC, P, S).transpose(1, 0, 2))
            )

    runner = _get_runner()
    concat = {
        "xt": np.concatenate(xt_cores, axis=0),
        "cst32": np.concatenate([cst32] * 8, axis=0),
        "cst16": np.concatenate([c16[c % 2] for c in range(8)], axis=0),
        "qpos": np.concatenate([qp[c % 2] for c in range(8)], axis=0),
    }
    for n, v in w16.items():
        concat[n] = np.concatenate([v] * 8, axis=0)
    return [concat[n] for n in runner["in_names"]]


def _run_concat(concat_in):
    runner = _get_runner()
    zeros = [
        np.zeros((8 * a.shape[0], *a.shape[1:]), a.dtype) for a in runner["out_avals"]
    ]
    out_arrs = runner["sharded"](*concat_in, *zeros)
    ot = np.asarray(out_arrs[runner["out_names"].index("ot")]).reshape(8, P, S)
    rs = np.asarray(out_arrs[runner["out_names"].index("rs")]).reshape(8, S)
    return ot, rs


def _assemble(ot, rs):
    bv = _NC_CACHE["bv"]
    slotperm = _NC_CACHE["slotperm"]
    out = np.empty((B, S, D), dtype=np.float32)
    for b in range(B):
        acc_o = np.zeros((D, S), np.float64)
        acc_r = np.zeros((S,), np.float64)
        for role in (0, 1):
            c = 2 * b + role
            perm = slotperm[role]
            acc_o[:, perm] += ot[c]
            acc_r[perm] += rs[c]
        out[b] = (acc_o / acc_r[None, :]).T + bv[None, :]
    return out


def kernel(x, wq, bq, wk, bk, wv, bv):
    concat_in = _prep_in_concat(x, wq, bq, wk, bk, wv, bv)
    ot, rs = _run_concat(concat_in)
    return _assemble(ot, rs)


# revision 36
# speedup vs baseline: 1.1308x; 1.0278x over previous
"""Causal single-head attention (B=4, S=2048, E=1024, D=128) on 8 trn2 cores.

KEY-SPLIT sharding: 2 cores per batch. Core role r owns the key tiles with
nat index == r (mod 2) (8 of 16 128-row tiles) and computes K/V only for
those; BOTH cores process ALL 2048 queries against their own keys
(flash-style partials), and the host combines:
    out = (otA + otB) / (rsA + rsB)  (+bv, transpose)

xt column layout per core (pos space): cols 0..1023 = own key tiles
ascending, cols 1024..2047 = the other role's tiles ascending. Query
slot s = pos-tiles {own 2s, own 2s+1, other 2s, other 2s+1} (512 queries),
so slot s needs exactly the local key tiles 0..2s+1 for BOTH roles:
20 score units per core, no padding waste. Only the last key-tile pair of
each slot can straddle the diagonal; it is masked with a position-compare
STT (qpos >= tpos, host tables, role-baked).

Device program (fp16 operands, f32 PSUM, exp pairs over [128,1024]):
  K^T, V from own xt cols; Q^T per slot from (own pair | other pair) cols
  slot s, key pair u: st[t,q] pair -> exp pair -> (last pair: STT mask)
  racc (+)= pt halves [DVE];  ot[d,q] += v.T @ pt  [PE]
  rs = ones.T@racc + direct ones.T@pt for the last pair
"""

import math

import numpy as np

B, S, E, D = 4, 2048, 1024, 128
P = 128
EC = E // P          # 8 E-chunks
NK = 8               # own key tiles per core
SCALE = 1.0 / math.sqrt(D)


def _pos_to_nat(role):
    """pos-tile -> nat-tile mapping per role."""
    own = [2 * t + role for t in range(NK)]
    other = [2 * t + 1 - role for t in range(NK)]
    return np.array(own + other)


def _build_nc():
    from contextlib import ExitStack

    import concourse.bass as bass
    import concourse.tile as tile
    from concourse import bacc, mybir

    f16 = mybir.dt.float16
    f32 = mybir.dt.float32
    AF = mybir.ActivationFunctionType

    nc = bacc.Bacc("TRN2", target_bir_lowering=False, debug=False)

    xt_in = nc.dram_tensor("xt", [P, EC, S], f16, kind="ExternalInput")
    w_in = {
        n: nc.dram_tensor(n, [P, EC, D], f16, kind="ExternalInput")
        for n in ("wq", "wk", "wv")
    }
    # cst32: col0 = bq, col1 = bk
    cst32_in = nc.dram_tensor("cst32", [P, 2], f32, kind="ExternalInput")
    # ones column + tpos (own key nat positions per local tile)
    cst16_in = nc.dram_tensor("cst16", [P, 1 + NK], f16, kind="ExternalInput")
    # qpos: nat position of the query in each SLOT column (replicated rows)
    qpos_in = nc.dram_tensor("qpos", [P, S], f16, kind="ExternalInput")
    ot_out = nc.dram_tensor("ot", [P, S], f32, kind="ExternalOutput")
    rs_out = nc.dram_tensor("rs", [1, S], f32, kind="ExternalOutput")

    with tile.TileContext(nc) as tc, ExitStack() as ctx:
        consts = ctx.enter_context(tc.tile_pool(name="consts", bufs=1))
        xt_pool = ctx.enter_context(tc.tile_pool(name="xt", bufs=4))
        pt_pool = ctx.enter_context(tc.tile_pool(name="pt", bufs=8))
        out_pool = ctx.enter_context(tc.tile_pool(name="outp", bufs=1))
        pj_psum = ctx.enter_context(tc.tile_pool(name="pjp", bufs=2, space="PSUM"))
        vv_psum = ctx.enter_context(tc.tile_pool(name="vvp", bufs=1, space="PSUM"))
        st_psum = ctx.enter_context(tc.tile_pool(name="stp", bufs=3, space="PSUM"))
        ot_psum = ctx.enter_context(tc.tile_pool(name="otp", bufs=1, space="PSUM"))

        w_sb = {}
        for n in ("wk", "wv", "wq"):
            w_sb[n] = consts.tile([P, EC, D], f16, name=f"w_{n}")
        xt_tiles = {}   # 4 column-quarters of 512
        for qb in range(4):
            xt_tiles[qb] = xt_pool.tile([P, EC, 512], f16, tag="xt", name=f"xt_{qb}")
        cst32 = consts.tile([P, 2], f32)
        cst16 = consts.tile([P, 1 + NK], f16)
        qpos_sb = consts.tile([P, S], f16)

        def ld(q, sb, dram, ch0, ch1, col0=None, col1=None):
            if col0 is None:
                q.dma_start(out=sb[:, ch0:ch1], in_=dram[:, ch0:ch1])
            else:
                q.dma_start(out=sb[:, ch0:ch1, :], in_=dram[:, ch0:ch1, col0:col1])

        # priority: the critical chain is xt2 -> Q0 -> slot0, and HWDGE
        # costs ~625ns per DMA on one queue — so merge the xt loads into
        # whole-tile DMAs and split the early ones across both queues.
        ld(nc.scalar, w_sb["wk"], w_in["wk"], 0, 2)
        ld(nc.sync, xt_tiles[0], xt_in, 0, 2, 0, 512)
        ld(nc.sync, w_sb["wk"], w_in["wk"], 2, EC)
        ld(nc.sync, xt_tiles[0], xt_in, 2, 4, 0, 512)
        ld(nc.sync, xt_tiles[0], xt_in, 4, EC, 0, 512)
        nc.sync.dma_start(out=cst32[:], in_=cst32_in[:, :])
        ld(nc.sync, w_sb["wq"], w_in["wq"], 0, EC)
        nc.sync.dma_start(out=cst16[:], in_=cst16_in[:, :])
        nc.sync.dma_start(out=qpos_sb[:, 0:512], in_=qpos_in[:, 0:512])
        for h in range(4):
            ld(nc.sync, xt_tiles[2], xt_in, h * 2, (h + 1) * 2, 1024, 1536)
        ld(nc.sync, w_sb["wv"], w_in["wv"], 0, EC)
        for h in range(2):
            ld(nc.sync, xt_tiles[1], xt_in, h * 4, (h + 1) * 4, 512, 1024)
        nc.sync.dma_start(out=qpos_sb[:, 512:2048], in_=qpos_in[:, 512:2048])
        for h in range(2):
            ld(nc.sync, xt_tiles[3], xt_in, h * 4, (h + 1) * 4, 1536, 2048)

        bq, bk = cst32[:, 0:1], cst32[:, 1:2]
        ones = cst16[:, 0:1]
        tpos = cst16[:, 1 : 1 + NK]

        kt_tiles = {}
        qt_tiles = {}
        v_big = consts.tile([P, NK, D], f16, name="v_big")

        def xcols(c, lo, hi):
            """xt slice for pos-cols [lo, hi) of chunk c."""
            qb, off = lo // 512, lo % 512
            assert hi <= (qb + 1) * 512
            return xt_tiles[qb][:, c, off : off + (hi - lo)]

        def proj_k_gen(g):
            # `warm` dummy matmuls on already-resident data are interleaved
            # between the chunk pairs: during the DMA-feed-limited start the
            # PE would otherwise idle >1us between chunks, resetting the
            # p-state ramp (matmuls then run at 0.65GHz instead of 2.4GHz).
            pp = pj_psum.tile([P, 512], f32, tag="pj")
            for c in range(EC):
                nc.tensor.matmul(
                    pp[:], w_sb["wk"][:, c, :], xcols(c, g * 512, (g + 1) * 512),
                    start=(c == 0), stop=(c == EC - 1),
                )
                if c % 2 == 1:
                    yield
            kt = consts.tile([P, 512], f16, name=f"kt_{g}")
            nc.vector.tensor_scalar_add(kt[:], pp[:], bk)
            kt_tiles[g] = kt
            yield

        def proj_v_gen(g):
            for jp in range(2):
                vp = vv_psum.tile([P, 2, D], f32, tag="vv")
                for h in range(2):
                    jl = g * 4 + jp * 2 + h
                    for c in range(EC):
                        nc.tensor.matmul(
                            vp[:, h, :],
                            xcols(c, jl * P, (jl + 1) * P),
                            w_sb["wv"][:, c, :],
                            start=(c == 0), stop=(c == EC - 1),
                        )
                    yield
                nc.vector.tensor_copy(
                    v_big[:, g * 4 + jp * 2 : g * 4 + jp * 2 + 2, :], vp[:]
                )

        def proj_q_gen(s):
            # slot s queries: own pair cols [256s, 256s+256) and other pair
            # cols [1024 + 256s, +256)
            pp = pj_psum.tile([P, 512], f32, tag="pj")
            for half, base in ((0, 256 * s), (1, 1024 + 256 * s)):
                for c in range(EC):
                    nc.tensor.matmul(
                        pp[:, half * 256 : half * 256 + 256],
                        w_sb["wq"][:, c, :], xcols(c, base, base + 256),
                        start=(c == 0), stop=(c == EC - 1),
                    )
                    if c % 2 == 1:
                        yield
            qt = consts.tile([P, 512], f16, name=f"qt_{s}")
            nc.vector.tensor_scalar_add(qt[:], pp[:], bq)
            qt_tiles[s] = qt
            yield

        ot_sb = out_pool.tile([P, S], f32)
        rs_sb = out_pool.tile([1, S], f32)

        def attn_slot_gen(s):
            n_pr = s + 1          # key pairs 0..s; last pair masked
            qt = qt_tiles[s]
            qp = None
            ot = ot_psum.tile([P, 512], f32, tag="ot")
            rp = ot_psum.tile([1, 512], f32, tag="rs")
            racc = out_pool.tile([P, 512], f16, name=f"racc_{s}")

            tail_trick = True        # direct-rs drain for every slot

            def emit_pv(seq, pos, ptp):
                for h in (0, 1):
                    nc.tensor.matmul(
                        ot[:], v_big[:, 2 * pos + h, :], ptp[:, h, :],
                        start=(seq == 0 and h == 0),
                        stop=(seq == n_pr - 1 and h == 1),
                    )
                if tail_trick and n_pr > 1 and seq == n_pr - 2:
                    nc.tensor.matmul(rp[:], ones, racc[:], start=True, stop=False)

            pend = []
            # masked (diagonal) pair first: its longer exp->STT chain then
            # overlaps the plain pairs' exps instead of sitting in the tail
            order = [n_pr - 1] + list(range(n_pr - 1))
            for seq, pos in enumerate(order):
                j0 = 2 * pos
                ptp = pt_pool.tile([P, 2, 512], f16, tag="pt")
                masked = pos == n_pr - 1
                final = seq == n_pr - 1
                for h in (0, 1):
                    j = j0 + h
                    sth = st_psum.tile([P, 512], f32, tag="st")
                    nc.tensor.matmul(
                        sth[:],
                        kt_tiles[j // 4][:, (j % 4) * P : (j % 4 + 1) * P],
                        qt[:], start=True, stop=True,
                    )
                    nc.scalar.activation(
                        out=ptp[:, h, :], in_=sth[:], func=AF.Exp, scale=SCALE,
                    )
                    if masked:
                        nc.vector.scalar_tensor_tensor(
                            out=ptp[:, h, :],
                            in0=qpos_sb[:, s * 512 : (s + 1) * 512],
                            scalar=tpos[:, j : j + 1],
                            in1=ptp[:, h, :],
                            op0=mybir.AluOpType.is_ge,
                            op1=mybir.AluOpType.mult,
                        )
                if not (final and tail_trick):
                    if seq == 0:
                        nc.vector.tensor_copy(racc[:], ptp[:, 0, :])
                        nc.vector.tensor_add(racc[:], racc[:], ptp[:, 1, :])
                    else:
                        nc.vector.tensor_add(racc[:], racc[:], ptp[:, 0, :])
                        nc.vector.tensor_add(racc[:], racc[:], ptp[:, 1, :])
                pend.append((seq, pos, ptp))
                yield
                if len(pend) > 2:
                    emit_pv(*pend.pop(0))
                yield
            while pend:
                seq, pos, ptp = pend.pop(0)
                if seq < n_pr - 1 or not tail_trick:
                    emit_pv(seq, pos, ptp)
                    if not tail_trick and seq == n_pr - 1:
                        nc.tensor.matmul(rp[:], ones, racc[:], start=True, stop=True)
                else:
                    for h in (0, 1):
                        nc.tensor.matmul(
                            ot[:], v_big[:, 2 * pos + h, :], ptp[:, h, :],
                            start=(seq == 0 and h == 0), stop=(h == 1),
                        )
                        nc.tensor.matmul(
                            rp[:], ones, ptp[:, h, :],
                            start=(n_pr == 1 and h == 0), stop=(h == 1),
                        )
            nc.vector.tensor_copy(ot_sb[:, s * 512 : s * 512 + 256], ot[:, 0:256])
            nc.scalar.copy(ot_sb[:, s * 512 + 256 : (s + 1) * 512], ot[:, 256:512])
            nc.vector.tensor_copy(rs_sb[0:1, s * 512 : (s + 1) * 512], rp[:])
            nc.sync.dma_start(
                out=rs_out[:, s * 512 : (s + 1) * 512],
                in_=rs_sb[0:1, s * 512 : (s + 1) * 512],
            )
            nc.sync.dma_start(
                out=ot_out[:, s * 512 : s * 512 + 256],
                in_=ot_sb[:, s * 512 : s * 512 + 256],
            )
            nc.scalar.dma_start(
                out=ot_out[:, s * 512 + 256 : (s + 1) * 512],
                in_=ot_sb[:, s * 512 + 256 : (s + 1) * 512],
            )

        def drain(gen):
            for _ in gen:
                pass

        def interleave(main, filler, ratio=1):
            for _ in main:
                for _ in range(ratio):
                    next(filler, None)

        def chain(*gens):
            for g in gens:
                yield from g

        # K for local tiles 0..3, Q0+Q1 up front, then the attention
        # cascade; later projections ride as fillers inside the windows so
        # each slot's qt is ready before its window opens.
        drain(proj_k_gen(0))
        drain(proj_q_gen(0))
        f1 = chain(proj_q_gen(1), proj_v_gen(0))
        interleave(attn_slot_gen(0), f1, ratio=7)
        drain(f1)
        f2 = chain(proj_k_gen(1), proj_q_gen(2), proj_v_gen(1))
        interleave(attn_slot_gen(1), f2, ratio=3)
        drain(f2)
        f3 = proj_q_gen(3)
        interleave(attn_slot_gen(2), f3, ratio=2)
        drain(f3)
        drain(attn_slot_gen(3))

    nc.compile()
    return nc


_NC_CACHE = {}


def _get_nc():
    if "nc" not in _NC_CACHE:
        _NC_CACHE["nc"] = _build_nc()
    return _NC_CACHE["nc"]


def _get_runner():
    if "runner" in _NC_CACHE:
        return _NC_CACHE["runner"]

    import jax
    from jax.sharding import Mesh, PartitionSpec
    from jax.experimental.shard_map import shard_map
    from concourse import bass2jax, mybir

    nc = _get_nc()
    bass2jax.install_neuronx_cc_hook()

    partition_name = nc.partition_id_tensor.name if nc.partition_id_tensor else None
    in_names, out_names, out_avals = [], [], []
    for alloc in nc.m.functions[0].allocations:
        if not isinstance(alloc, mybir.MemoryLocationSet):
            continue
        name = alloc.memorylocations[0].name
        if alloc.kind == "ExternalInput":
            if name != partition_name:
                in_names.append(name)
        elif alloc.kind == "ExternalOutput":
            out_names.append(name)
            out_avals.append(
                jax.core.ShapedArray(tuple(alloc.tensor_shape), mybir.dt.np(alloc.dtype))
            )
    n_params = len(in_names)
    all_names = in_names + out_names
    if partition_name is not None:
        all_names = all_names + [partition_name]

    def _body(*args):
        operands = list(args)
        if partition_name is not None:
            operands.append(bass2jax.partition_id_tensor())
        outs = bass2jax._bass_exec_p.bind(
            *operands,
            out_avals=tuple(out_avals),
            in_names=tuple(all_names),
            out_names=tuple(out_names),
            lowering_input_output_aliases=(),
            sim_require_finite=True,
            sim_require_nnan=True,
            nc=nc,
        )
        return tuple(outs)

    devices = jax.devices()[:8]
    mesh = Mesh(np.asarray(devices), ("core",))
    sharded = jax.jit(
        shard_map(
            _body,
            mesh=mesh,
            in_specs=(PartitionSpec("core"),) * (n_params + len(out_names)),
            out_specs=(PartitionSpec("core"),) * len(out_names),
            check_rep=False,
        ),
        donate_argnums=tuple(range(n_params, n_params + len(out_names))),
        keep_unused=True,
    )
    runner = {
        "sharded": sharded,
        "in_names": in_names,
        "out_names": out_names,
        "out_avals": out_avals,
    }
    _NC_CACHE["runner"] = runner
    return runner


def _prep_in_concat(x, wq, bq, wk, bk, wv, bv):
    x = np.asarray(x, dtype=np.float32)

    if "pos2nat" not in _NC_CACHE:
        _NC_CACHE["pos2nat"] = [_pos_to_nat(0), _pos_to_nat(1)]
    p2n = _NC_CACHE["pos2nat"]

    def pack_w(w):
        return np.ascontiguousarray(
            np.asarray(w, np.float32).reshape(EC, P, D).transpose(1, 0, 2)
        ).astype(np.float16)

    w16 = {"wq": pack_w(wq), "wk": pack_w(wk), "wv": pack_w(wv)}
    _NC_CACHE["bv"] = np.asarray(bv, np.float32)

    cst32 = np.stack(
        [np.asarray(bq, np.float32), np.asarray(bk, np.float32)], axis=1
    )  # [P, 2]

    c16, qp = [], []
    for role in (0, 1):
        nat = p2n[role]
        t = np.empty((P, 1 + NK), np.float16)
        t[:, 0] = 1.0
        for j in range(NK):
            t[:, 1 + j] = (nat[j] * P + np.arange(P)).astype(np.float16)
        c16.append(t)
        colpos = (
            nat[:, None] * P + np.arange(P)[None, :]
        ).reshape(-1)                              # nat position of pos-col
        # slot order: slot s = own pair cols | other pair cols
        slotpos = np.concatenate(
            [np.concatenate([colpos[256 * s : 256 * s + 256],
                             colpos[1024 + 256 * s : 1024 + 256 * s + 256]])
             for s in range(4)]
        )
        qp.append(
            np.ascontiguousarray(
                np.tile(slotpos[None, :].astype(np.float16), (P, 1))
            )
        )
    _NC_CACHE["slotperm"] = []
    for r in (0, 1):
        colpos = (p2n[r][:, None] * P + np.arange(P)[None, :]).reshape(-1)
        _NC_CACHE["slotperm"].append(
            np.concatenate(
                [np.concatenate([colpos[256 * s : 256 * s + 256],
                                 colpos[1024 + 256 * s : 1024 + 256 * s + 256]])
                 for s in range(4)]
            )
        )
    _NC_CACHE["colperm"] = [
        (p2n[r][:, None] * P + np.arange(P)[None, :]).reshape(-1) for r in (0, 1)
    ]

    xt_cores = []
    for b in range(B):
        xbT = np.ascontiguousarray(x[b].T)  # [E, S]
        for role in (0, 1):
            xg = xbT[:, _NC_CACHE["colperm"][role]].astype(np.float16)
            xt_cores.append(
                np.ascontiguousarray(xg.reshape(EC, P, S).transpose(1, 0, 2))
            )

    runner = _get_runner()
    concat = {
        "xt": np.concatenate(xt_cores, axis=0),
        "cst32": np.concatenate([cst32] * 8, axis=0),
        "cst16": np.concatenate([c16[c % 2] for c in range(8)], axis=0),
        "qpos": np.concatenate([qp[c % 2] for c in range(8)], axis=0),
    }
    for n, v in w16.items():
        concat[n] = np.concatenate([v] * 8, axis=0)
    return [concat[n] for n in runner["in_names"]]


def _run_concat(concat_in):
    runner = _get_runner()
    zeros = [
        np.zeros((8 * a.shape[0], *a.shape[1:]), a.dtype) for a in runner["out_avals"]
    ]
    out_arrs = runner["sharded"](*concat_in, *zeros)
    ot = np.asarray(out_arrs[runner["out_names"].index("ot")]).reshape(8, P, S)
    rs = np.asarray(out_arrs[runner["out_names"].index("rs")]).reshape(8, S)
    return ot, rs


def _assemble(ot, rs):
    bv = _NC_CACHE["bv"]
    slotperm = _NC_CACHE["slotperm"]
    out = np.empty((B, S, D), dtype=np.float32)
    for b in range(B):
        acc_o = np.zeros((D, S), np.float64)
        acc_r = np.zeros((S,), np.float64)
        for role in (0, 1):
            c = 2 * b + role
            perm = slotperm[role]
            acc_o[:, perm] += ot[c]
            acc_r[perm] += rs[c]
        out[b] = (acc_o / acc_r[None, :]).T + bv[None, :]
    return out


def kernel(x, wq, bq, wk, bk, wv, bv):
    concat_in = _prep_in_concat(x, wq, bq, wk, bk, wv, bv)
    ot, rs = _run_concat(concat_in)
    return _assemble(ot, rs)


# revision 37
# speedup vs baseline: 1.1371x; 1.0055x over previous
"""Causal single-head attention (B=4, S=2048, E=1024, D=128) on 8 trn2 cores.

KEY-SPLIT sharding: 2 cores per batch. Core role r owns the key tiles with
nat index == r (mod 2) (8 of 16 128-row tiles) and computes K/V only for
those; BOTH cores process ALL 2048 queries against their own keys
(flash-style partials), and the host combines:
    out = (otA + otB) / (rsA + rsB)  (+bv, transpose)

xt column layout per core (pos space): cols 0..1023 = own key tiles
ascending, cols 1024..2047 = the other role's tiles ascending. Query
slot s = pos-tiles {own 2s, own 2s+1, other 2s, other 2s+1} (512 queries),
so slot s needs exactly the local key tiles 0..2s+1 for BOTH roles:
20 score units per core, no padding waste. Only the last key-tile pair of
each slot can straddle the diagonal; it is masked with a position-compare
STT (qpos >= tpos, host tables, role-baked).

Device program (fp16 operands, f32 PSUM, exp pairs over [128,1024]):
  K^T, V from own xt cols; Q^T per slot from (own pair | other pair) cols
  slot s, key pair u: st[t,q] pair -> exp pair -> (last pair: STT mask)
  racc (+)= pt halves [DVE];  ot[d,q] += v.T @ pt  [PE]
  rs = ones.T@racc + direct ones.T@pt for the last pair
"""

import math

import numpy as np

B, S, E, D = 4, 2048, 1024, 128
P = 128
EC = E // P          # 8 E-chunks
NK = 8               # own key tiles per core
SCALE = 1.0 / math.sqrt(D)


def _pos_to_nat(role):
    """pos-tile -> nat-tile mapping per role."""
    own = [2 * t + role for t in range(NK)]
    other = [2 * t + 1 - role for t in range(NK)]
    return np.array(own + other)


def _build_nc():
    from contextlib import ExitStack

    import concourse.bass as bass
    import concourse.tile as tile
    from concourse import bacc, mybir

    f16 = mybir.dt.float16
    f32 = mybir.dt.float32
    AF = mybir.ActivationFunctionType

    nc = bacc.Bacc("TRN2", target_bir_lowering=False, debug=False)

    xt_in = nc.dram_tensor("xt", [P, EC, S], f16, kind="ExternalInput")
    w_in = {
        n: nc.dram_tensor(n, [P, EC, D], f16, kind="ExternalInput")
        for n in ("wq", "wk", "wv")
    }
    # cst32: col0 = bq, col1 = bk
    cst32_in = nc.dram_tensor("cst32", [P, 2], f32, kind="ExternalInput")
    # ones column + tpos (own key nat positions per local tile)
    cst16_in = nc.dram_tensor("cst16", [P, 1 + NK], f16, kind="ExternalInput")
    # qpos: nat position of the query in each SLOT column (replicated rows)
    qpos_in = nc.dram_tensor("qpos", [P, S], f16, kind="ExternalInput")
    ot_out = nc.dram_tensor("ot", [P, S], f32, kind="ExternalOutput")
    rs_out = nc.dram_tensor("rs", [1, S], f32, kind="ExternalOutput")

    with tile.TileContext(nc) as tc, ExitStack() as ctx:
        consts = ctx.enter_context(tc.tile_pool(name="consts", bufs=1))
        xt_pool = ctx.enter_context(tc.tile_pool(name="xt", bufs=4))
        pt_pool = ctx.enter_context(tc.tile_pool(name="pt", bufs=8))
        out_pool = ctx.enter_context(tc.tile_pool(name="outp", bufs=1))
        pj_psum = ctx.enter_context(tc.tile_pool(name="pjp", bufs=2, space="PSUM"))
        vv_psum = ctx.enter_context(tc.tile_pool(name="vvp", bufs=1, space="PSUM"))
        st_psum = ctx.enter_context(tc.tile_pool(name="stp", bufs=3, space="PSUM"))
        ot_psum = ctx.enter_context(tc.tile_pool(name="otp", bufs=1, space="PSUM"))

        w_sb = {}
        for n in ("wk", "wv", "wq"):
            w_sb[n] = consts.tile([P, EC, D], f16, name=f"w_{n}")
        xt_tiles = {}   # 4 column-quarters of 512
        for qb in range(4):
            xt_tiles[qb] = xt_pool.tile([P, EC, 512], f16, tag="xt", name=f"xt_{qb}")
        cst32 = consts.tile([P, 2], f32)
        cst16 = consts.tile([P, 1 + NK], f16)
        qpos_sb = consts.tile([P, S], f16)

        def ld(q, sb, dram, ch0, ch1, col0=None, col1=None):
            if col0 is None:
                q.dma_start(out=sb[:, ch0:ch1], in_=dram[:, ch0:ch1])
            else:
                q.dma_start(out=sb[:, ch0:ch1, :], in_=dram[:, ch0:ch1, col0:col1])

        # priority: the critical chain is xt2 -> Q0 -> slot0, and HWDGE
        # costs ~625ns per DMA on one queue — so merge the xt loads into
        # whole-tile DMAs and split the early ones across both queues.
        ld(nc.scalar, w_sb["wk"], w_in["wk"], 0, 2)
        ld(nc.sync, xt_tiles[0], xt_in, 0, 2, 0, 512)
        ld(nc.sync, w_sb["wk"], w_in["wk"], 2, EC)
        ld(nc.sync, xt_tiles[0], xt_in, 2, 4, 0, 512)
        ld(nc.sync, xt_tiles[0], xt_in, 4, EC, 0, 512)
        nc.sync.dma_start(out=cst32[:], in_=cst32_in[:, :])
        ld(nc.sync, w_sb["wq"], w_in["wq"], 0, EC)
        nc.sync.dma_start(out=cst16[:], in_=cst16_in[:, :])
        nc.sync.dma_start(out=qpos_sb[:, 0:512], in_=qpos_in[:, 0:512])
        for h in range(4):
            ld(nc.sync, xt_tiles[2], xt_in, h * 2, (h + 1) * 2, 1024, 1536)
        ld(nc.sync, w_sb["wv"], w_in["wv"], 0, EC)
        for h in range(2):
            ld(nc.sync, xt_tiles[1], xt_in, h * 4, (h + 1) * 4, 512, 1024)
        nc.sync.dma_start(out=qpos_sb[:, 512:2048], in_=qpos_in[:, 512:2048])
        for h in range(2):
            ld(nc.sync, xt_tiles[3], xt_in, h * 4, (h + 1) * 4, 1536, 2048)

        bq, bk = cst32[:, 0:1], cst32[:, 1:2]
        ones = cst16[:, 0:1]
        tpos = cst16[:, 1 : 1 + NK]

        kt_tiles = {}
        qt_tiles = {}
        v_big = consts.tile([P, NK, D], f16, name="v_big")

        def xcols(c, lo, hi):
            """xt slice for pos-cols [lo, hi) of chunk c."""
            qb, off = lo // 512, lo % 512
            assert hi <= (qb + 1) * 512
            return xt_tiles[qb][:, c, off : off + (hi - lo)]

        def proj_k_gen(g):
            # `warm` dummy matmuls on already-resident data are interleaved
            # between the chunk pairs: during the DMA-feed-limited start the
            # PE would otherwise idle >1us between chunks, resetting the
            # p-state ramp (matmuls then run at 0.65GHz instead of 2.4GHz).
            pp = pj_psum.tile([P, 512], f32, tag="pj")
            for c in range(EC):
                nc.tensor.matmul(
                    pp[:], w_sb["wk"][:, c, :], xcols(c, g * 512, (g + 1) * 512),
                    start=(c == 0), stop=(c == EC - 1),
                )
                if c % 2 == 1:
                    yield
            kt = consts.tile([P, 512], f16, name=f"kt_{g}")
            nc.vector.tensor_scalar_add(kt[:], pp[:], bk)
            kt_tiles[g] = kt
            yield

        def proj_v_gen(g):
            for jp in range(2):
                vp = vv_psum.tile([P, 2, D], f32, tag="vv")
                for h in range(2):
                    jl = g * 4 + jp * 2 + h
                    for c in range(EC):
                        nc.tensor.matmul(
                            vp[:, h, :],
                            xcols(c, jl * P, (jl + 1) * P),
                            w_sb["wv"][:, c, :],
                            start=(c == 0), stop=(c == EC - 1),
                        )
                    yield
                nc.vector.tensor_copy(
                    v_big[:, g * 4 + jp * 2 : g * 4 + jp * 2 + 2, :], vp[:]
                )

        def proj_q_gen(s):
            # slot s queries: own pair cols [256s, 256s+256) and other pair
            # cols [1024 + 256s, +256)
            pp = pj_psum.tile([P, 512], f32, tag="pj")
            for half, base in ((0, 256 * s), (1, 1024 + 256 * s)):
                for c in range(EC):
                    nc.tensor.matmul(
                        pp[:, half * 256 : half * 256 + 256],
                        w_sb["wq"][:, c, :], xcols(c, base, base + 256),
                        start=(c == 0), stop=(c == EC - 1),
                    )
                    if c % 2 == 1:
                        yield
            qt = consts.tile([P, 512], f16, name=f"qt_{s}")
            nc.vector.tensor_scalar_add(qt[:], pp[:], bq)
            qt_tiles[s] = qt
            yield

        ot_sb = out_pool.tile([P, S], f32)
        rs_sb = out_pool.tile([1, S], f32)

        def attn_slot_gen(s):
            n_pr = s + 1          # key pairs 0..s; last pair masked
            qt = qt_tiles[s]
            qp = None
            ot = ot_psum.tile([P, 512], f32, tag="ot")
            rp = ot_psum.tile([1, 512], f32, tag="rs")
            racc = out_pool.tile([P, 512], f16, name=f"racc_{s}")

            tail_trick = True        # direct-rs drain for every slot

            def emit_pv(seq, pos, ptp):
                for h in (0, 1):
                    nc.tensor.matmul(
                        ot[:], v_big[:, 2 * pos + h, :], ptp[:, h, :],
                        start=(seq == 0 and h == 0),
                        stop=(seq == n_pr - 1 and h == 1),
                    )
                if tail_trick and n_pr > 1 and seq == n_pr - 2:
                    nc.tensor.matmul(rp[:], ones, racc[:], start=True, stop=False)

            pend = []
            # masked (diagonal) pair first: its longer exp->STT chain then
            # overlaps the plain pairs' exps instead of sitting in the tail
            order = [n_pr - 1] + list(range(n_pr - 1))
            for seq, pos in enumerate(order):
                j0 = 2 * pos
                ptp = pt_pool.tile([P, 2, 512], f16, tag="pt")
                masked = pos == n_pr - 1
                final = seq == n_pr - 1
                for h in (0, 1):
                    j = j0 + h
                    sth = st_psum.tile([P, 512], f32, tag="st")
                    nc.tensor.matmul(
                        sth[:],
                        kt_tiles[j // 4][:, (j % 4) * P : (j % 4 + 1) * P],
                        qt[:], start=True, stop=True,
                    )
                    nc.scalar.activation(
                        out=ptp[:, h, :], in_=sth[:], func=AF.Exp, scale=SCALE,
                    )
                    if masked:
                        nc.vector.scalar_tensor_tensor(
                            out=ptp[:, h, :],
                            in0=qpos_sb[:, s * 512 : (s + 1) * 512],
                            scalar=tpos[:, j : j + 1],
                            in1=ptp[:, h, :],
                            op0=mybir.AluOpType.is_ge,
                            op1=mybir.AluOpType.mult,
                        )
                if not (final and tail_trick):
                    if seq == 0:
                        nc.vector.tensor_copy(racc[:], ptp[:, 0, :])
                        nc.vector.tensor_add(racc[:], racc[:], ptp[:, 1, :])
                    else:
                        nc.vector.tensor_add(racc[:], racc[:], ptp[:, 0, :])
                        nc.vector.tensor_add(racc[:], racc[:], ptp[:, 1, :])
                pend.append((seq, pos, ptp))
                yield
                if len(pend) > 2:
                    emit_pv(*pend.pop(0))
                yield
            while pend:
                seq, pos, ptp = pend.pop(0)
                if seq < n_pr - 1 or not tail_trick:
                    emit_pv(seq, pos, ptp)
                    if not tail_trick and seq == n_pr - 1:
                        nc.tensor.matmul(rp[:], ones, racc[:], start=True, stop=True)
                else:
                    for h in (0, 1):
                        nc.tensor.matmul(
                            ot[:], v_big[:, 2 * pos + h, :], ptp[:, h, :],
                            start=(seq == 0 and h == 0), stop=(h == 1),
                        )
                        nc.tensor.matmul(
                            rp[:], ones, ptp[:, h, :],
                            start=(n_pr == 1 and h == 0), stop=(h == 1),
                        )
            nc.vector.tensor_copy(ot_sb[:, s * 512 : s * 512 + 256], ot[:, 0:256])
            nc.scalar.copy(ot_sb[:, s * 512 + 256 : (s + 1) * 512], ot[:, 256:512])
            nc.vector.tensor_copy(rs_sb[0:1, s * 512 : (s + 1) * 512], rp[:])
            nc.sync.dma_start(
                out=rs_out[:, s * 512 : (s + 1) * 512],
                in_=rs_sb[0:1, s * 512 : (s + 1) * 512],
            )
            nc.sync.dma_start(
                out=ot_out[:, s * 512 : s * 512 + 256],
                in_=ot_sb[:, s * 512 : s * 512 + 256],
            )
            oq = nc.scalar if s == 3 else nc.sync
            oq.dma_start(
                out=ot_out[:, s * 512 + 256 : (s + 1) * 512],
                in_=ot_sb[:, s * 512 + 256 : (s + 1) * 512],
            )

        def drain(gen):
            for _ in gen:
                pass

        def interleave(main, filler, ratio=1):
            for _ in main:
                for _ in range(ratio):
                    next(filler, None)

        def chain(*gens):
            for g in gens:
                yield from g

        # K for local tiles 0..3, Q0+Q1 up front, then the attention
        # cascade; later projections ride as fillers inside the windows so
        # each slot's qt is ready before its window opens.
        drain(proj_k_gen(0))
        drain(proj_q_gen(0))
        f1 = chain(proj_q_gen(1), proj_v_gen(0))
        interleave(attn_slot_gen(0), f1, ratio=7)
        drain(f1)
        f2 = chain(proj_k_gen(1), proj_q_gen(2), proj_v_gen(1))
        interleave(attn_slot_gen(1), f2, ratio=3)
        drain(f2)
        f3 = proj_q_gen(3)
        interleave(attn_slot_gen(2), f3, ratio=2)
        drain(f3)
        drain(attn_slot_gen(3))

    nc.compile()
    return nc


_NC_CACHE = {}


def _get_nc():
    if "nc" not in _NC_CACHE:
        _NC_CACHE["nc"] = _build_nc()
    return _NC_CACHE["nc"]


def _get_runner():
    if "runner" in _NC_CACHE:
        return _NC_CACHE["runner"]

    import jax
    from jax.sharding import Mesh, PartitionSpec
    from jax.experimental.shard_map import shard_map
    from concourse import bass2jax, mybir

    nc = _get_nc()
    bass2jax.install_neuronx_cc_hook()

    partition_name = nc.partition_id_tensor.name if nc.partition_id_tensor else None
    in_names, out_names, out_avals = [], [], []
    for alloc in nc.m.functions[0].allocations:
        if not isinstance(alloc, mybir.MemoryLocationSet):
            continue
        name = alloc.memorylocations[0].name
        if alloc.kind == "ExternalInput":
            if name != partition_name:
                in_names.append(name)
        elif alloc.kind == "ExternalOutput":
            out_names.append(name)
            out_avals.append(
                jax.core.ShapedArray(tuple(alloc.tensor_shape), mybir.dt.np(alloc.dtype))
            )
    n_params = len(in_names)
    all_names = in_names + out_names
    if partition_name is not None:
        all_names = all_names + [partition_name]

    def _body(*args):
        operands = list(args)
        if partition_name is not None:
            operands.append(bass2jax.partition_id_tensor())
        outs = bass2jax._bass_exec_p.bind(
            *operands,
            out_avals=tuple(out_avals),
            in_names=tuple(all_names),
            out_names=tuple(out_names),
            lowering_input_output_aliases=(),
            sim_require_finite=True,
            sim_require_nnan=True,
            nc=nc,
        )
        return tuple(outs)

    devices = jax.devices()[:8]
    mesh = Mesh(np.asarray(devices), ("core",))
    sharded = jax.jit(
        shard_map(
            _body,
            mesh=mesh,
            in_specs=(PartitionSpec("core"),) * (n_params + len(out_names)),
            out_specs=(PartitionSpec("core"),) * len(out_names),
            check_rep=False,
        ),
        donate_argnums=tuple(range(n_params, n_params + len(out_names))),
        keep_unused=True,
    )
    runner = {
        "sharded": sharded,
        "in_names": in_names,
        "out_names": out_names,
        "out_avals": out_avals,
    }
    _NC_CACHE["runner"] = runner
    return runner


def _prep_in_concat(x, wq, bq, wk, bk, wv, bv):
    x = np.asarray(x, dtype=np.float32)

    if "pos2nat" not in _NC_CACHE:
        _NC_CACHE["pos2nat"] = [_pos_to_nat(0), _pos_to_nat(1)]
    p2n = _NC_CACHE["pos2nat"]

    def pack_w(w):
        return np.ascontiguousarray(
            np.asarray(w, np.float32).reshape(EC, P, D).transpose(1, 0, 2)
        ).astype(np.float16)

    w16 = {"wq": pack_w(wq), "wk": pack_w(wk), "wv": pack_w(wv)}
    _NC_CACHE["bv"] = np.asarray(bv, np.float32)

    cst32 = np.stack(
        [np.asarray(bq, np.float32), np.asarray(bk, np.float32)], axis=1
    )  # [P, 2]

    c16, qp = [], []
    for role in (0, 1):
        nat = p2n[role]
        t = np.empty((P, 1 + NK), np.float16)
        t[:, 0] = 1.0
        for j in range(NK):
            t[:, 1 + j] = (nat[j] * P + np.arange(P)).astype(np.float16)
        c16.append(t)
        colpos = (
            nat[:, None] * P + np.arange(P)[None, :]
        ).reshape(-1)                              # nat position of pos-col
        # slot order: slot s = own pair cols | other pair cols
        slotpos = np.concatenate(
            [np.concatenate([colpos[256 * s : 256 * s + 256],
                             colpos[1024 + 256 * s : 1024 + 256 * s + 256]])
             for s in range(4)]
        )
        qp.append(
            np.ascontiguousarray(
                np.tile(slotpos[None, :].astype(np.float16), (P, 1))
            )
        )
    _NC_CACHE["slotperm"] = []
    for r in (0, 1):
        colpos = (p2n[r][:, None] * P + np.arange(P)[None, :]).reshape(-1)
        _NC_CACHE["slotperm"].append(
            np.concatenate(
                [np.concatenate([colpos[256 * s : 256 * s + 256],
                                 colpos[1024 + 256 * s : 1024 + 256 * s + 256]])
                 for s in range(4)]
            )
        )
    _NC_CACHE["colperm"] = [
        (p2n[r][:, None] * P + np.arange(P)[None, :]).reshape(-1) for r in (0, 1)
    ]

    xt_cores = []
    for b in range(B):
        xbT = np.ascontiguousarray(x[b].T)  # [E, S]
        for role in (0, 1):
            xg = xbT[:, _NC_CACHE["colperm"][role]].astype(np.float16)
            xt_cores.append(
                np.ascontiguousarray(xg.reshape(EC, P, S).transpose(1, 0, 2))
            )

    runner = _get_runner()
    concat = {
        "xt": np.concatenate(xt_cores, axis=0),
        "cst32": np.concatenate([cst32] * 8, axis=0),
        "cst16": np.concatenate([c16[c % 2] for c in range(8)], axis=0),
        "qpos": np.concatenate([qp[c % 2] for c in range(8)], axis=0),
    }
    for n, v in w16.items():
        concat[n] = np.concatenate([v] * 8, axis=0)
    return [concat[n] for n in runner["in_names"]]


def _run_concat(concat_in):
    runner = _get_runner()
    zeros = [
        np.zeros((8 * a.shape[0], *a.shape[1:]), a.dtype) for a in runner["out_avals"]
    ]
    out_arrs = runner["sharded"](*concat_in, *zeros)
    ot = np.asarray(out_arrs[runner["out_names"].index("ot")]).reshape(8, P, S)
    rs = np.asarray(out_arrs[runner["out_names"].index("rs")]).reshape(8, S)
    return ot, rs


def _assemble(ot, rs):
    bv = _NC_CACHE["bv"]
    slotperm = _NC_CACHE["slotperm"]
    out = np.empty((B, S, D), dtype=np.float32)
    for b in range(B):
        acc_o = np.zeros((D, S), np.float64)
        acc_r = np.zeros((S,), np.float64)
        for role in (0, 1):
            c = 2 * b + role
            perm = slotperm[role]
            acc_o[:, perm] += ot[c]
            acc_r[perm] += rs[c]
        out[b] = (acc_o / acc_r[None, :]).T + bv[None, :]
    return out


def kernel(x, wq, bq, wk, bk, wv, bv):
    concat_in = _prep_in_concat(x, wq, bq, wk, bk, wv, bv)
    ot, rs = _run_concat(concat_in)
    return _assemble(ot, rs)
